# revision 11
# baseline (speedup 1.0000x reference)
"""Sparse-attention kernel for 8 trn2 NeuronCores (Bass/Tile) — v6.

Math (reference):
    Q = x1 @ Wq.T + bq                       [N1, DIM]
    K = x2 @ Wk.T + bk                       [N2, DIM]
    scores = (Q @ K.T) / sqrt(ITEM)          [N1, N2]
    e = exp(scores) * label_map
    att = e / (sum_j e + 1e-8) * (sum_j label_map / topk + 1e-8)
    out = att @ x2                           [N1, ITEM]

Key transformations (on top of the v5 baseline):
  * Rows of x1/label_map sharded across 8 cores (512 rows each); bk drops
    out of the normalization (scales numerator and denominator equally).
  * Each core projects only its own 512-column shard of K.T; the shard is
    AllGathered in TWO d-halves so the first collective launches mid-way
    through the K projection and both finish under the Q projection.
  * Scores are computed TRANSPOSED (eT tiles [n2-rows, own-rows]) by
    swapping matmul operands: weights = K.T chunks, moving = Q.T rows.
    This removes all 128 PE transposes + 128 scalar copies of v5: the exp
    output lands directly in the spmm operand layout.
  * label masking is folded into the exp argument: host ships
    M = (label-1)*30 and the kernel computes e = exp(scores + M), so the
    masked entries underflow to ~e-26 (negligible vs sums ~1e3).
  * Row sums of e (the softmax denominator) use a ones-weight matmul chain
    accumulated across all 32 eT tiles; interactions = rowsum(label) is
    reduced on the (otherwise idle) vector engine from a second,
    untransposed copy of the label map during the Q projection.
  * spmm processes output columns in PAIRS: each et weight tile is loaded
    once per 2 matmuls (1024 PE cycles), halving LDWEIGHTS traffic; the 8
    PSUM banks hold 4 row-chunks x 2 column-tiles of accumulators, and
    drains are split across the vector and scalar engines.
  * DMA streams are spread over 4 queues (sync/scalar/vector/gpsimd) and
    issued in ~256-512KB pieces so the first K-proj matmul starts ~5us in
    and no phase waits on a single-queue stream (~125GB/s per queue).
  * Matmul operands are bf16 (fp32 PSUM accumulation); weights partition-
    major so every SBUF slab loads with fully contiguous multi-KB lines.
"""

import math

import numpy as np

try:
    import concourse.bass as bass
except ImportError:  # fresh interpreter without the boot path
    import sys

    sys.path.insert(0, "/opt/trn_rl_repo")
    import concourse.bass as bass

import ml_dtypes
import concourse.mybir as mybir
import concourse.tile as tile
from concourse import bacc
from concourse.bass_utils import run_bass_kernel_spmd

NCORES = 8
F32 = mybir.dt.float32
BF16 = mybir.dt.bfloat16
NPBF16 = ml_dtypes.bfloat16


def _build(S, N2, ITEM, DIMP, denom, topk_f):
    """Build the per-core Bass program.

    S     - x1 rows per core (multiple of 128)
    N2    - x2 rows (multiple of 512)
    ITEM  - feature dim (multiple of 512)
    DIMP  - projection dim padded to a multiple of 128
    denom - sqrt(original ITEM)
    """
    IC = S // 128  # own-row chunks
    JC = N2 // 128  # x2-row chunks (spmm contraction, eT partition blocks)
    JN = N2 // 512  # 512-wide blocks of x2 rows (one per core's K shard)
    TC = ITEM // 128  # feature chunks (projection contraction)
    TN = ITEM // 512  # 512-wide tiles of the output free dim
    DC = DIMP // 128  # projection-dim chunks
    DH = DC // 2  # d-chunks per AllGather half
    assert JN == NCORES and S == 512
    Exp = mybir.ActivationFunctionType.Exp
    Mult = mybir.AluOpType.mult
    Add = mybir.AluOpType.add
    X = mybir.AxisListType.X

    nc = bacc.Bacc("TRN2", target_bir_lowering=False, debug=False, num_devices=NCORES)
    x1t = nc.dram_tensor("x1t", [128, TC, S], BF16, kind="ExternalInput")
    wqt = nc.dram_tensor("wqt", [DC, 128, TC, 128], BF16, kind="ExternalInput")
    wkt = nc.dram_tensor("wkt", [DC, 128, TC, 128], BF16, kind="ExternalInput")
    x2m = nc.dram_tensor("x2m", [128, TC, 512], BF16, kind="ExternalInput")
    x2n = nc.dram_tensor("x2n", [TN, 128, JC, 512], BF16, kind="ExternalInput")
    lmt = nc.dram_tensor("lmt", [128, JC, S], BF16, kind="ExternalInput")
    lmn = nc.dram_tensor("lmn", [JN, 128, IC, 512], BF16, kind="ExternalInput")
    bq2 = nc.dram_tensor("bq2", [128, DC], F32, kind="ExternalInput")
    y = nc.dram_tensor("y", [S, ITEM], F32, kind="ExternalOutput")

    with tile.TileContext(nc) as tc:
        with (
            tc.tile_pool(name="big", bufs=1) as big,
            tc.tile_pool(name="persist", bufs=1) as persist,
            tc.tile_pool(name="s8k", bufs=16) as s8k,
            tc.tile_pool(name="lmtp", bufs=8) as lmtp,
            tc.tile_pool(name="lmnp", bufs=2) as lmnp,
            tc.tile_pool(name="outp", bufs=4) as outp,
            tc.tile_pool(name="dram", bufs=1, space="DRAM") as drampool,
            tc.tile_pool(name="acc", bufs=8, space="PSUM") as accp,
        ):
            ones_b = persist.tile([128, 1], BF16, tag="onesb")
            nc.gpsimd.memset(ones_b[:], 1.0)
            ones_f = persist.tile([128, 1], F32, tag="onesf")
            nc.gpsimd.memset(ones_f[:], 1.0)
            bqs = persist.tile([128, DC], F32, tag="bqs")
            nc.sync.dma_start(bqs[:], bq2[:])

            # ---- phase A: own K.T shard projection, chunked AllGather ----
            # DMA queues (only sync/scalar/gpsimd can issue): the first K-proj
            # matmul is gated by wk0's first chunk (gpsimd) + x2m slab 0's
            # first half (scalar), each 512KB, so PE starts ~5us in.  Per-
            # engine emission order is chosen so every stream lands just
            # ahead of its consumer at ~125GB/s per queue.
            wk = []
            for d in range(DC):
                wk.append(s8k.tile([128, TC, 128], BF16, tag="s8", name=f"wk_{d}"))
            nc.gpsimd.dma_start(wk[0][:, 0:16, :], wkt[0, :, 0:16, :])
            nc.gpsimd.dma_start(wk[0][:, 16:32, :], wkt[0, :, 16:32, :])
            # x2m as whole slabs (8KB/partition lines): 2 on scalar, 2 on sync
            xm = []
            for q in range(4):
                xm.append(s8k.tile([128, 8, 512], BF16, tag="s8", name=f"x2m_{q}"))
                eng = nc.scalar if q < 2 else nc.sync
                eng.dma_start(xm[q][:], x2m[:, q * 8 : (q + 1) * 8, :])
            # wk0/1/2/4/6 on gpsimd (d0/d1 gate the K-proj pipeline); odd
            # stragglers go to scalar behind the early x1 chunks
            for d in (1, 2, 4, 6):
                nc.gpsimd.dma_start(wk[d][:], wkt[d])
            x1s = big.tile([128, TC, S], BF16, tag="bigA", name="x1s")
            for q in (0, 2):
                nc.scalar.dma_start(
                    x1s[:, q * 4 : (q + 1) * 4, :], x1t[:, q * 4 : (q + 1) * 4, :]
                )
            for d in (3, 5, 7):
                nc.scalar.dma_start(wk[d][:], wkt[d])
            for q in (1, 3, 5, 7):
                nc.sync.dma_start(
                    x1s[:, q * 4 : (q + 1) * 4, :], x1t[:, q * 4 : (q + 1) * 4, :]
                )
            ktsb = persist.tile([128, DC, 512], BF16, tag="ktsb")
            # 3-way chunked AllGather: launch after d1 / d4 / d7 so the gather
            # bandwidth overlaps the rest of phase A and the Q projection
            CCS = [(0, 2), (2, 5), (5, 8)]
            ktin = [
                drampool.tile(
                    [128, hi - lo, 512], BF16, tag=f"ktin{h}", name=f"ktin{h}"
                )
                for h, (lo, hi) in enumerate(CCS)
            ]
            ktall = [
                drampool.tile(
                    [NCORES, 128, hi - lo, 512], BF16, tag=f"ktall{h}",
                    name=f"ktall{h}", addr_space="Shared",
                )
                for h, (lo, hi) in enumerate(CCS)
            ]
            for d in range(DC):
                ps = accp.tile([128, 512], F32, tag="acc", name=f"psk_{d}")
                for t in range(TC):
                    nc.tensor.matmul(
                        ps[:],
                        wk[d][:, t, :],
                        xm[t // 8][:, t % 8, :],
                        start=(t == 0),
                        stop=(t == TC - 1),
                    )
                nc.scalar.copy(ktsb[:, d, :], ps[:])
                for h, (lo, hi) in enumerate(CCS):
                    if d == hi - 1:
                        nc.scalar.dma_start(ktin[h][:], ktsb[:, lo:hi, :])
                        nc.gpsimd.collective_compute(
                            "AllGather",
                            mybir.AluOpType.bypass,
                            replica_groups=[list(range(NCORES))],
                            ins=[ktin[h][:].opt()],
                            outs=[ktall[h][:].opt()],
                        )
                if d == 2:
                    for q in (4, 6):
                        nc.scalar.dma_start(
                            x1s[:, q * 4 : (q + 1) * 4, :],
                            x1t[:, q * 4 : (q + 1) * 4, :],
                        )

            # ---- phase 1: QT[d, i] = ((x1 @ Wq.T) + bq) / denom, DIM-major ----
            qt = persist.tile([128, DC, S], BF16, tag="qt")
            for d in range(DC):
                wsl = s8k.tile([128, TC, 128], BF16, tag="s8", name=f"wq_{d}")
                weng = nc.gpsimd if d % 2 == 0 else nc.sync
                weng.dma_start(wsl[:], wqt[d])
                ps = accp.tile([128, 512], F32, tag="acc", name=f"psq_{d}")
                for t in range(TC):
                    nc.tensor.matmul(
                        ps[:],
                        wsl[:, t, :],
                        x1s[:, t, :],
                        start=(t == 0),
                        stop=(t == TC - 1),
                    )
                nc.vector.tensor_scalar(
                    qt[:, d, :], ps[:], 1.0 / denom, bqs[:, d : d + 1],
                    op0=Mult, op1=Add,
                )

            # ---- interactions = rowsum(label), on the idle vector engine ----
            iparts = persist.tile([128, IC, JN], F32, tag="iparts")
            for jn in range(JN):
                lsl = lmnp.tile([128, IC, 512], BF16, tag="lmn")
                nc.sync.dma_start(lsl[:], lmn[jn])
                for i in range(IC):
                    nc.vector.reduce_sum(iparts[:, i, jn : jn + 1], lsl[:, i, :], axis=X)
            ia = persist.tile([128, IC, 1], F32, tag="ia")
            nc.vector.reduce_sum(ia[:], iparts[:], axis=X)
            nc.vector.tensor_scalar(ia[:], ia[:], 1.0 / topk_f, 1e-8, op0=Mult, op1=Add)

            # ---- phase 3: transposed scores -> exp -> eT tiles + e row-sums ----
            et = big.tile([128, JC, S], BF16, tag="bigA", name="et")
            esum = accp.tile([1, 512], F32, tag="acc", name="esum")
            # the e row-sum matmuls lag 2 tiles behind the exp pipeline so the
            # PE never waits on the vector-add + exp epilogue of the same tile
            pend = []

            def flush_esum(limit):
                while len(pend) > limit:
                    j0 = pend.pop(0)
                    nc.tensor.matmul(
                        esum[:], ones_b[:], et[:, j0, :],
                        start=(j0 == 0), stop=(j0 == JC - 1),
                    )

            ch_first = []
            for jn in range(JN):
                kt = s8k.tile([128, DC, 512], BF16, tag="s8", name=f"kt_{jn}")
                nc.gpsimd.dma_start(kt[:, 0:2, :], ktall[0][jn])
                nc.gpsimd.dma_start(kt[:, 2:5, :], ktall[1][jn])
                nc.gpsimd.dma_start(kt[:, 5:8, :], ktall[2][jn])
                if jn == 2:
                    # n=0 spmm chunks: queued behind kt_0..2 so the K.T tiles
                    # win the queue race; they land during phase 3.
                    for jq in range(4):
                        c = s8k.tile(
                            [128, 8, 512], BF16, tag="s8", name=f"x2c_0_{jq}"
                        )
                        nc.scalar.dma_start(c[:], x2n[0, :, jq * 8 : (jq + 1) * 8, :])
                        ch_first.append(c)
                for jl in range(4):
                    jj = jn * 4 + jl
                    lmc = lmtp.tile([128, 1, 512], BF16, tag="lmt")
                    nc.scalar.dma_start(lmc[:], lmt[:, jj : jj + 1, :])
                    ps = accp.tile([128, 512], F32, tag="acc", name=f"ps3_{jj}")
                    for d in range(DC):
                        nc.tensor.matmul(
                            ps[:],
                            kt[:, d, jl * 128 : (jl + 1) * 128],
                            qt[:, d, :],
                            start=(d == 0),
                            stop=(d == DC - 1),
                        )
                    nc.vector.tensor_add(ps[:], ps[:], lmc[:, 0, :])
                    nc.scalar.activation(et[:, jj, :], ps[:], Exp)
                    pend.append(jj)
                    flush_esum(2)
            flush_esum(0)

            # ---- a_i = (interactions/topk + 1e-8) / (sum_e + 1e-8), column-major ----
            esr = persist.tile([1, 512], F32, tag="esr")
            nc.scalar.copy(esr[:], esum[:])
            ecol = persist.tile([128, IC, 1], F32, tag="ecol")
            for i in range(IC):
                pt = accp.tile([128, 1], F32, tag="acc", name=f"tr_{i}")
                nc.tensor.transpose(
                    pt[:], esr[:, i * 128 : (i + 1) * 128], ones_f[0:1, 0:1]
                )
                nc.scalar.copy(ecol[:, i, :], pt[:])
            rec = persist.tile([128, IC, 1], F32, tag="rec")
            nc.vector.tensor_scalar_add(ecol[:], ecol[:], 1e-8)
            nc.vector.reciprocal(rec[:], ecol[:])
            asb = persist.tile([128, IC, 1], F32, tag="asb")
            nc.vector.tensor_mul(asb[:], ia[:], rec[:])

            # ---- phase 4: spmm, one 32-matmul chain per output tile ----
            for n in range(TN):
                if n == 0:
                    ch = ch_first
                else:
                    ch = []
                    for jq in range(4):
                        c = s8k.tile([128, 8, 512], BF16, tag="s8", name=f"x2c_{n}_{jq}")
                        eng = nc.scalar if jq % 2 == 0 else nc.gpsimd
                        eng.dma_start(c[:], x2n[n, :, jq * 8 : (jq + 1) * 8, :])
                        ch.append(c)
                for i in range(IC):
                    ps = accp.tile([128, 512], F32, tag="acc", name=f"ps4_{n}_{i}")
                    for j in range(JC):
                        nc.tensor.matmul(
                            ps[:],
                            et[:, j, i * 128 : (i + 1) * 128],
                            ch[j // 8][:, j % 8, :],
                            start=(j == 0),
                            stop=(j == JC - 1),
                        )
                    o = outp.tile([128, 512], F32, tag="o")
                    if i % 2 == 0:
                        nc.vector.tensor_scalar_mul(o[:], ps[:], asb[:, i, :])
                    else:
                        nc.scalar.mul(o[:], ps[:], asb[:, i, :])
                    nc.sync.dma_start(
                        y[i * 128 : (i + 1) * 128, n * 512 : (n + 1) * 512], o[:]
                    )

    nc.compile()
    return nc


def _pmajor(a, p, inner):
    """[R, C] with R = nblk*p -> [p, nblk, C] partition-major, where each
    partition's inner block is contiguous."""
    R, C = a.shape
    nblk = R // p
    return np.ascontiguousarray(a.reshape(nblk, p, C).transpose(1, 0, 2))


def _in_maps(x1, x2, label_map, Wq, bq, Wk, DIMP, S, denom):
    ITEM = x1.shape[1]
    N2 = x2.shape[0]
    DIM = Wq.shape[0]
    DC = DIMP // 128
    TC = ITEM // 128
    JN = N2 // 512
    TN = ITEM // 512
    JC = N2 // 128
    IC = S // 128

    wqp = np.zeros((DIMP, ITEM), NPBF16)
    wqp[:DIM] = Wq.astype(NPBF16)
    wkp = np.zeros((DIMP, ITEM), NPBF16)
    wkp[:DIM] = Wk.astype(NPBF16)
    bqp = np.zeros((DIMP,), np.float32)
    bqp[:DIM] = bq / denom
    bq2 = np.ascontiguousarray(bqp.reshape(DC, 128).T)

    x1b = x1.astype(NPBF16)
    x2b = x2.astype(NPBF16)
    wqT = np.ascontiguousarray(wqp.T)  # [ITEM, DIMP]
    x2T = np.ascontiguousarray(x2b.T)  # [ITEM, N2]

    # wqt[d] = WqT[:, d-chunk] as [128, TC, 128] partition-major
    wqt = np.stack(
        [_pmajor(wqT[:, d * 128 : (d + 1) * 128], 128, None) for d in range(DC)]
    )
    wkT = np.ascontiguousarray(wkp.T)
    wktb = np.stack(
        [_pmajor(wkT[:, d * 128 : (d + 1) * 128], 128, None) for d in range(DC)]
    )
    # x2t[jn] = x2T[:, jn-chunk] as [128, TC, 512]
    x2tb = np.stack(
        [_pmajor(x2T[:, j * 512 : (j + 1) * 512], 128, None) for j in range(JN)]
    )
    # x2n[n] = x2[:, n-chunk] as [128, JC, 512]
    x2nb = np.stack(
        [_pmajor(x2b[:, n * 512 : (n + 1) * 512], 128, None) for n in range(TN)]
    )
    maps = []
    for c in range(NCORES):
        sl = slice(c * S, (c + 1) * S)
        shard = label_map[sl]
        # normal orientation (0/1) for interaction row sums
        lmb = np.stack(
            [
                _pmajor(shard.astype(NPBF16)[:, j * 512 : (j + 1) * 512], 128, None)
                for j in range(JN)
            ]
        )
        # transposed additive mask: 0 where label=1, -30 where label=0
        mt = ((shard.T.astype(np.float32) - 1.0) * 30.0).astype(NPBF16)  # [N2, S]
        lmtb = _pmajor(mt, 128, None)  # [128, JC, S]
        maps.append(
            {
                "x1t": _pmajor(np.ascontiguousarray(x1b[sl].T), 128, None),
                "wqt": wqt,
                "wkt": wktb,
                "x2m": x2tb[c],
                "x2n": x2nb,
                "lmt": lmtb,
                "lmn": lmb,
                "bq2": bq2,
            }
        )
    return maps


def _run(x1, x2, label_map, Wq, bq, Wk, bk, topk, trace=False):
    x1 = np.asarray(x1, np.float32)
    x2 = np.asarray(x2, np.float32)
    label_map = np.asarray(label_map, np.float32)
    Wq = np.asarray(Wq, np.float32)
    bq = np.asarray(bq, np.float32)
    Wk = np.asarray(Wk, np.float32)
    N1, ITEM = x1.shape
    N2 = x2.shape[0]
    DIM = Wq.shape[0]
    S = N1 // NCORES
    DIMP = ((DIM + 127) // 128) * 128
    denom = math.sqrt(ITEM)
    nc = _build(S, N2, ITEM, DIMP, denom, float(topk))
    maps = _in_maps(x1, x2, label_map, Wq, bq, Wk, DIMP, S, denom)
    res = run_bass_kernel_spmd(
        nc, maps, list(range(NCORES)), trace=trace, trace_cores=[0] if trace else None
    )
    out = np.concatenate([res.results[c]["y"] for c in range(NCORES)], axis=0)
    return out.astype(np.float32), res


def kernel(x1, x2, label_map, Wq, bq, Wk, bk, topk):
    out, _ = _run(x1, x2, label_map, Wq, bq, Wk, bk, topk)
    return out


# revision 13
# speedup vs baseline: 1.1065x; 1.1065x over previous
"""Sparse-attention kernel for 8 trn2 NeuronCores (Bass/Tile) — v6.

Math (reference):
    Q = x1 @ Wq.T + bq                       [N1, DIM]
    K = x2 @ Wk.T + bk                       [N2, DIM]
    scores = (Q @ K.T) / sqrt(ITEM)          [N1, N2]
    e = exp(scores) * label_map
    att = e / (sum_j e + 1e-8) * (sum_j label_map / topk + 1e-8)
    out = att @ x2                           [N1, ITEM]

Key transformations (on top of the v5 baseline):
  * Rows of x1/label_map sharded across 8 cores (512 rows each); bk drops
    out of the normalization (scales numerator and denominator equally).
  * Each core projects only its own 512-column shard of K.T; the shard is
    AllGathered in TWO d-halves so the first collective launches mid-way
    through the K projection and both finish under the Q projection.
  * Scores are computed TRANSPOSED (eT tiles [n2-rows, own-rows]) by
    swapping matmul operands: weights = K.T chunks, moving = Q.T rows.
    This removes all 128 PE transposes + 128 scalar copies of v5: the exp
    output lands directly in the spmm operand layout.
  * label masking is folded into the exp argument: host ships
    M = (label-1)*30 and the kernel computes e = exp(scores + M), so the
    masked entries underflow to ~e-26 (negligible vs sums ~1e3).
  * Row sums of e (the softmax denominator) use a ones-weight matmul chain
    accumulated across all 32 eT tiles; interactions = rowsum(label) is
    reduced on the (otherwise idle) vector engine from a second,
    untransposed copy of the label map during the Q projection.
  * spmm processes output columns in PAIRS: each et weight tile is loaded
    once per 2 matmuls (1024 PE cycles), halving LDWEIGHTS traffic; the 8
    PSUM banks hold 4 row-chunks x 2 column-tiles of accumulators, and
    drains are split across the vector and scalar engines.
  * DMA streams are spread over 4 queues (sync/scalar/vector/gpsimd) and
    issued in ~256-512KB pieces so the first K-proj matmul starts ~5us in
    and no phase waits on a single-queue stream (~125GB/s per queue).
  * Matmul operands are bf16 (fp32 PSUM accumulation); weights partition-
    major so every SBUF slab loads with fully contiguous multi-KB lines.
"""

import math

import numpy as np

try:
    import concourse.bass as bass
except ImportError:  # fresh interpreter without the boot path
    import sys

    sys.path.insert(0, "/opt/trn_rl_repo")
    import concourse.bass as bass

import ml_dtypes
import concourse.mybir as mybir
import concourse.tile as tile
from concourse import bacc
from concourse.bass_utils import run_bass_kernel_spmd

NCORES = 8
F32 = mybir.dt.float32
BF16 = mybir.dt.bfloat16
NPBF16 = ml_dtypes.bfloat16


def _build(S, N2, ITEM, DIMP, denom, topk_f):
    """Build the per-core Bass program.

    S     - x1 rows per core (multiple of 128)
    N2    - x2 rows (multiple of 512)
    ITEM  - feature dim (multiple of 512)
    DIMP  - projection dim padded to a multiple of 128
    denom - sqrt(original ITEM)
    """
    IC = S // 128  # own-row chunks
    JC = N2 // 128  # x2-row chunks (spmm contraction, eT partition blocks)
    JN = N2 // 512  # 512-wide blocks of x2 rows (one per core's K shard)
    TC = ITEM // 128  # feature chunks (projection contraction)
    TN = ITEM // 512  # 512-wide tiles of the output free dim
    DC = DIMP // 128  # projection-dim chunks
    DH = DC // 2  # d-chunks per AllGather half
    assert JN == NCORES and S == 512
    Exp = mybir.ActivationFunctionType.Exp
    Mult = mybir.AluOpType.mult
    Add = mybir.AluOpType.add
    X = mybir.AxisListType.X

    nc = bacc.Bacc("TRN2", target_bir_lowering=False, debug=False, num_devices=NCORES)
    x1t = nc.dram_tensor("x1t", [128, TC, S], BF16, kind="ExternalInput")
    wqt = nc.dram_tensor("wqt", [DC, 128, TC, 128], BF16, kind="ExternalInput")
    wkt = nc.dram_tensor("wkt", [DC, 128, TC, 128], BF16, kind="ExternalInput")
    x2m = nc.dram_tensor("x2m", [128, TC, 512], BF16, kind="ExternalInput")
    x2n = nc.dram_tensor("x2n", [TN, 128, JC, 512], BF16, kind="ExternalInput")
    lmt = nc.dram_tensor("lmt", [128, JC, S], BF16, kind="ExternalInput")
    lmn = nc.dram_tensor("lmn", [JN, 128, IC, 512], BF16, kind="ExternalInput")
    bq2 = nc.dram_tensor("bq2", [128, DC], F32, kind="ExternalInput")
    y = nc.dram_tensor("y", [S, ITEM], F32, kind="ExternalOutput")

    with tile.TileContext(nc) as tc:
        with (
            tc.tile_pool(name="big", bufs=1) as big,
            tc.tile_pool(name="persist", bufs=1) as persist,
            tc.tile_pool(name="s8k", bufs=16) as s8k,
            tc.tile_pool(name="lmtp", bufs=8) as lmtp,
            tc.tile_pool(name="lmnp", bufs=2) as lmnp,
            tc.tile_pool(name="outp", bufs=4) as outp,
            tc.tile_pool(name="dram", bufs=1, space="DRAM") as drampool,
            tc.tile_pool(name="acc", bufs=8, space="PSUM") as accp,
        ):
            ones_b = persist.tile([128, 1], BF16, tag="onesb")
            nc.gpsimd.memset(ones_b[:], 1.0)
            ones_f = persist.tile([128, 1], F32, tag="onesf")
            nc.gpsimd.memset(ones_f[:], 1.0)
            bqs = persist.tile([128, DC], F32, tag="bqs")
            nc.sync.dma_start(bqs[:], bq2[:])

            # ---- phase A: own K.T shard projection, chunked AllGather ----
            # DMA queues (only sync/scalar/gpsimd can issue): the first K-proj
            # matmul is gated by wk0's first chunk (gpsimd) + x2m slab 0's
            # first half (scalar), each 512KB, so PE starts ~5us in.  Per-
            # engine emission order is chosen so every stream lands just
            # ahead of its consumer at ~125GB/s per queue.
            wk = []
            for d in range(DC):
                wk.append(s8k.tile([128, TC, 128], BF16, tag="s8", name=f"wk_{d}"))
            nc.gpsimd.dma_start(wk[0][:, 0:16, :], wkt[0, :, 0:16, :])
            nc.gpsimd.dma_start(wk[0][:, 16:32, :], wkt[0, :, 16:32, :])
            # x2m as whole slabs (8KB/partition lines): 2 on scalar, 2 on sync
            xm = []
            for q in range(4):
                xm.append(s8k.tile([128, 8, 512], BF16, tag="s8", name=f"x2m_{q}"))
                eng = nc.scalar if q < 2 else nc.sync
                eng.dma_start(xm[q][:], x2m[:, q * 8 : (q + 1) * 8, :])
            # wk0/1/2/4/6 on gpsimd (d0/d1 gate the K-proj pipeline); odd
            # stragglers go to scalar behind the early x1 chunks
            for d in (1, 2, 4, 6):
                nc.gpsimd.dma_start(wk[d][:], wkt[d])
            x1s = big.tile([128, TC, S], BF16, tag="bigA", name="x1s")
            for q in (0, 2):
                nc.scalar.dma_start(
                    x1s[:, q * 4 : (q + 1) * 4, :], x1t[:, q * 4 : (q + 1) * 4, :]
                )
            for d in (3, 5, 7):
                nc.scalar.dma_start(wk[d][:], wkt[d])
            for q in (1, 3, 5, 7):
                nc.sync.dma_start(
                    x1s[:, q * 4 : (q + 1) * 4, :], x1t[:, q * 4 : (q + 1) * 4, :]
                )
            ktsb = persist.tile([128, DC, 512], BF16, tag="ktsb")
            # 3-way chunked AllGather: launch after d1 / d4 / d7 so the gather
            # bandwidth overlaps the rest of phase A and the Q projection
            CCS = [(0, 2), (2, 5), (5, 8)]
            ktin = [
                drampool.tile(
                    [128, hi - lo, 512], BF16, tag=f"ktin{h}", name=f"ktin{h}"
                )
                for h, (lo, hi) in enumerate(CCS)
            ]
            ktall = [
                drampool.tile(
                    [NCORES, 128, hi - lo, 512], BF16, tag=f"ktall{h}",
                    name=f"ktall{h}", addr_space="Shared",
                )
                for h, (lo, hi) in enumerate(CCS)
            ]
            for d in range(DC):
                ps = accp.tile([128, 512], F32, tag="acc", name=f"psk_{d}")
                for t in range(TC):
                    nc.tensor.matmul(
                        ps[:],
                        wk[d][:, t, :],
                        xm[t // 8][:, t % 8, :],
                        start=(t == 0),
                        stop=(t == TC - 1),
                    )
                nc.scalar.copy(ktsb[:, d, :], ps[:])
                for h, (lo, hi) in enumerate(CCS):
                    if d == hi - 1:
                        nc.scalar.dma_start(ktin[h][:], ktsb[:, lo:hi, :])
                        nc.gpsimd.collective_compute(
                            "AllGather",
                            mybir.AluOpType.bypass,
                            replica_groups=[list(range(NCORES))],
                            ins=[ktin[h][:].opt()],
                            outs=[ktall[h][:].opt()],
                        )
                if d == 2:
                    for q in (4, 6):
                        nc.scalar.dma_start(
                            x1s[:, q * 4 : (q + 1) * 4, :],
                            x1t[:, q * 4 : (q + 1) * 4, :],
                        )

            # ---- phase 1: QT[d, i] = ((x1 @ Wq.T) + bq) / denom, DIM-major ----
            qt = persist.tile([128, DC, S], BF16, tag="qt")
            for d in range(DC):
                wsl = s8k.tile([128, TC, 128], BF16, tag="s8", name=f"wq_{d}")
                nc.gpsimd.dma_start(wsl[:], wqt[d])
                ps = accp.tile([128, 512], F32, tag="acc", name=f"psq_{d}")
                for t in range(TC):
                    nc.tensor.matmul(
                        ps[:],
                        wsl[:, t, :],
                        x1s[:, t, :],
                        start=(t == 0),
                        stop=(t == TC - 1),
                    )
                nc.vector.tensor_scalar(
                    qt[:, d, :], ps[:], 1.0 / denom, bqs[:, d : d + 1],
                    op0=Mult, op1=Add,
                )

            # ---- interactions = rowsum(label), on the idle vector engine ----
            iparts = persist.tile([128, IC, JN], F32, tag="iparts")
            for jn in range(JN):
                lsl = lmnp.tile([128, IC, 512], BF16, tag="lmn")
                nc.sync.dma_start(lsl[:], lmn[jn])
                for i in range(IC):
                    nc.vector.reduce_sum(iparts[:, i, jn : jn + 1], lsl[:, i, :], axis=X)
            ia = persist.tile([128, IC, 1], F32, tag="ia")
            nc.vector.reduce_sum(ia[:], iparts[:], axis=X)
            nc.vector.tensor_scalar(ia[:], ia[:], 1.0 / topk_f, 1e-8, op0=Mult, op1=Add)

            # ---- phase 3: transposed scores -> exp -> eT tiles + e row-sums ----
            et = big.tile([128, JC, S], BF16, tag="bigA", name="et")
            esum = accp.tile([1, 512], F32, tag="acc", name="esum")
            # the e row-sum matmuls lag 2 tiles behind the exp pipeline so the
            # PE never waits on the vector-add + exp epilogue of the same tile
            pend = []

            def flush_esum(limit):
                while len(pend) > limit:
                    j0 = pend.pop(0)
                    nc.tensor.matmul(
                        esum[:], ones_b[:], et[:, j0, :],
                        start=(j0 == 0), stop=(j0 == JC - 1),
                    )

            ch_first = []
            for jn in range(JN):
                lmcs = []
                for jl in range(4):
                    jj = jn * 4 + jl
                    lmc = lmtp.tile([128, 1, 512], BF16, tag="lmt", name=f"lm_{jj}")
                    nc.scalar.dma_start(lmc[:], lmt[:, jj : jj + 1, :])
                    lmcs.append(lmc)
                kt = s8k.tile([128, DC, 512], BF16, tag="s8", name=f"kt_{jn}")
                nc.sync.dma_start(kt[:, 0:2, :], ktall[0][jn])
                nc.sync.dma_start(kt[:, 2:5, :], ktall[1][jn])
                nc.scalar.dma_start(kt[:, 5:8, :], ktall[2][jn])
                if jn == 2:
                    # n=0 spmm chunks: queued behind kt_0..2 so the K.T tiles
                    # win the queue race; they land during phase 3.
                    for jq in range(4):
                        c = s8k.tile(
                            [128, 8, 512], BF16, tag="s8", name=f"x2c_0_{jq}"
                        )
                        nc.scalar.dma_start(c[:], x2n[0, :, jq * 8 : (jq + 1) * 8, :])
                        ch_first.append(c)
                for jl in range(4):
                    jj = jn * 4 + jl
                    lmc = lmcs[jl]
                    ps = accp.tile([128, 512], F32, tag="acc", name=f"ps3_{jj}")
                    for d in range(DC):
                        nc.tensor.matmul(
                            ps[:],
                            kt[:, d, jl * 128 : (jl + 1) * 128],
                            qt[:, d, :],
                            start=(d == 0),
                            stop=(d == DC - 1),
                        )
                    nc.vector.tensor_add(ps[:], ps[:], lmc[:, 0, :])
                    nc.scalar.activation(et[:, jj, :], ps[:], Exp)
                    pend.append(jj)
                    flush_esum(2)
            flush_esum(0)

            # ---- a_i = (interactions/topk + 1e-8) / (sum_e + 1e-8), column-major ----
            esr = persist.tile([1, 512], F32, tag="esr")
            nc.scalar.copy(esr[:], esum[:])
            ecol = persist.tile([128, IC, 1], F32, tag="ecol")
            for i in range(IC):
                pt = accp.tile([128, 1], F32, tag="acc", name=f"tr_{i}")
                nc.tensor.transpose(
                    pt[:], esr[:, i * 128 : (i + 1) * 128], ones_f[0:1, 0:1]
                )
                nc.scalar.copy(ecol[:, i, :], pt[:])
            rec = persist.tile([128, IC, 1], F32, tag="rec")
            nc.vector.tensor_scalar_add(ecol[:], ecol[:], 1e-8)
            nc.vector.reciprocal(rec[:], ecol[:])
            asb = persist.tile([128, IC, 1], F32, tag="asb")
            nc.vector.tensor_mul(asb[:], ia[:], rec[:])

            # ---- phase 4: spmm, one 32-matmul chain per output tile ----
            for n in range(TN):
                if n == 0:
                    ch = ch_first
                else:
                    ch = []
                    for jq in range(4):
                        c = s8k.tile([128, 8, 512], BF16, tag="s8", name=f"x2c_{n}_{jq}")
                        eng = nc.scalar if jq % 2 == 0 else nc.gpsimd
                        eng.dma_start(c[:], x2n[n, :, jq * 8 : (jq + 1) * 8, :])
                        ch.append(c)
                for i in range(IC):
                    ps = accp.tile([128, 512], F32, tag="acc", name=f"ps4_{n}_{i}")
                    for j in range(JC):
                        nc.tensor.matmul(
                            ps[:],
                            et[:, j, i * 128 : (i + 1) * 128],
                            ch[j // 8][:, j % 8, :],
                            start=(j == 0),
                            stop=(j == JC - 1),
                        )
                    o = outp.tile([128, 512], F32, tag="o")
                    if i % 2 == 0:
                        nc.vector.tensor_scalar_mul(o[:], ps[:], asb[:, i, :])
                    else:
                        nc.scalar.mul(o[:], ps[:], asb[:, i, :])
                    nc.sync.dma_start(
                        y[i * 128 : (i + 1) * 128, n * 512 : (n + 1) * 512], o[:]
                    )

    nc.compile()
    return nc


def _pmajor(a, p, inner):
    """[R, C] with R = nblk*p -> [p, nblk, C] partition-major, where each
    partition's inner block is contiguous."""
    R, C = a.shape
    nblk = R // p
    return np.ascontiguousarray(a.reshape(nblk, p, C).transpose(1, 0, 2))


def _in_maps(x1, x2, label_map, Wq, bq, Wk, DIMP, S, denom):
    ITEM = x1.shape[1]
    N2 = x2.shape[0]
    DIM = Wq.shape[0]
    DC = DIMP // 128
    TC = ITEM // 128
    JN = N2 // 512
    TN = ITEM // 512
    JC = N2 // 128
    IC = S // 128

    wqp = np.zeros((DIMP, ITEM), NPBF16)
    wqp[:DIM] = Wq.astype(NPBF16)
    wkp = np.zeros((DIMP, ITEM), NPBF16)
    wkp[:DIM] = Wk.astype(NPBF16)
    bqp = np.zeros((DIMP,), np.float32)
    bqp[:DIM] = bq / denom
    bq2 = np.ascontiguousarray(bqp.reshape(DC, 128).T)

    x1b = x1.astype(NPBF16)
    x2b = x2.astype(NPBF16)
    wqT = np.ascontiguousarray(wqp.T)  # [ITEM, DIMP]
    x2T = np.ascontiguousarray(x2b.T)  # [ITEM, N2]

    # wqt[d] = WqT[:, d-chunk] as [128, TC, 128] partition-major
    wqt = np.stack(
        [_pmajor(wqT[:, d * 128 : (d + 1) * 128], 128, None) for d in range(DC)]
    )
    wkT = np.ascontiguousarray(wkp.T)
    wktb = np.stack(
        [_pmajor(wkT[:, d * 128 : (d + 1) * 128], 128, None) for d in range(DC)]
    )
    # x2t[jn] = x2T[:, jn-chunk] as [128, TC, 512]
    x2tb = np.stack(
        [_pmajor(x2T[:, j * 512 : (j + 1) * 512], 128, None) for j in range(JN)]
    )
    # x2n[n] = x2[:, n-chunk] as [128, JC, 512]
    x2nb = np.stack(
        [_pmajor(x2b[:, n * 512 : (n + 1) * 512], 128, None) for n in range(TN)]
    )
    maps = []
    for c in range(NCORES):
        sl = slice(c * S, (c + 1) * S)
        shard = label_map[sl]
        # normal orientation (0/1) for interaction row sums
        lmb = np.stack(
            [
                _pmajor(shard.astype(NPBF16)[:, j * 512 : (j + 1) * 512], 128, None)
                for j in range(JN)
            ]
        )
        # transposed additive mask: 0 where label=1, -30 where label=0
        mt = ((shard.T.astype(np.float32) - 1.0) * 30.0).astype(NPBF16)  # [N2, S]
        lmtb = _pmajor(mt, 128, None)  # [128, JC, S]
        maps.append(
            {
                "x1t": _pmajor(np.ascontiguousarray(x1b[sl].T), 128, None),
                "wqt": wqt,
                "wkt": wktb,
                "x2m": x2tb[c],
                "x2n": x2nb,
                "lmt": lmtb,
                "lmn": lmb,
                "bq2": bq2,
            }
        )
    return maps


def _run(x1, x2, label_map, Wq, bq, Wk, bk, topk, trace=False):
    x1 = np.asarray(x1, np.float32)
    x2 = np.asarray(x2, np.float32)
    label_map = np.asarray(label_map, np.float32)
    Wq = np.asarray(Wq, np.float32)
    bq = np.asarray(bq, np.float32)
    Wk = np.asarray(Wk, np.float32)
    N1, ITEM = x1.shape
    N2 = x2.shape[0]
    DIM = Wq.shape[0]
    S = N1 // NCORES
    DIMP = ((DIM + 127) // 128) * 128
    denom = math.sqrt(ITEM)
    nc = _build(S, N2, ITEM, DIMP, denom, float(topk))
    maps = _in_maps(x1, x2, label_map, Wq, bq, Wk, DIMP, S, denom)
    res = run_bass_kernel_spmd(
        nc, maps, list(range(NCORES)), trace=trace, trace_cores=[0] if trace else None
    )
    out = np.concatenate([res.results[c]["y"] for c in range(NCORES)], axis=0)
    return out.astype(np.float32), res


def kernel(x1, x2, label_map, Wq, bq, Wk, bk, topk):
    out, _ = _run(x1, x2, label_map, Wq, bq, Wk, bk, topk)
    return out


# revision 14
# speedup vs baseline: 1.1383x; 1.0288x over previous
"""Sparse-attention kernel for 8 trn2 NeuronCores (Bass/Tile) — v6.

Math (reference):
    Q = x1 @ Wq.T + bq                       [N1, DIM]
    K = x2 @ Wk.T + bk                       [N2, DIM]
    scores = (Q @ K.T) / sqrt(ITEM)          [N1, N2]
    e = exp(scores) * label_map
    att = e / (sum_j e + 1e-8) * (sum_j label_map / topk + 1e-8)
    out = att @ x2                           [N1, ITEM]

Key transformations (on top of the v5 baseline):
  * Rows of x1/label_map sharded across 8 cores (512 rows each); bk drops
    out of the normalization (scales numerator and denominator equally).
  * Each core projects only its own 512-column shard of K.T; the shard is
    AllGathered in TWO d-halves so the first collective launches mid-way
    through the K projection and both finish under the Q projection.
  * Scores are computed TRANSPOSED (eT tiles [n2-rows, own-rows]) by
    swapping matmul operands: weights = K.T chunks, moving = Q.T rows.
    This removes all 128 PE transposes + 128 scalar copies of v5: the exp
    output lands directly in the spmm operand layout.
  * label masking is folded into the exp argument: host ships
    M = (label-1)*30 and the kernel computes e = exp(scores + M), so the
    masked entries underflow to ~e-26 (negligible vs sums ~1e3).
  * Row sums of e (the softmax denominator) use a ones-weight matmul chain
    accumulated across all 32 eT tiles; interactions = rowsum(label) is
    reduced on the (otherwise idle) vector engine from a second,
    untransposed copy of the label map during the Q projection.
  * spmm processes output columns in PAIRS: each et weight tile is loaded
    once per 2 matmuls (1024 PE cycles), halving LDWEIGHTS traffic; the 8
    PSUM banks hold 4 row-chunks x 2 column-tiles of accumulators, and
    drains are split across the vector and scalar engines.
  * DMA streams are spread over 4 queues (sync/scalar/vector/gpsimd) and
    issued in ~256-512KB pieces so the first K-proj matmul starts ~5us in
    and no phase waits on a single-queue stream (~125GB/s per queue).
  * Matmul operands are bf16 (fp32 PSUM accumulation); weights partition-
    major so every SBUF slab loads with fully contiguous multi-KB lines.
"""

import math

import numpy as np

try:
    import concourse.bass as bass
except ImportError:  # fresh interpreter without the boot path
    import sys

    sys.path.insert(0, "/opt/trn_rl_repo")
    import concourse.bass as bass

import ml_dtypes
import concourse.mybir as mybir
import concourse.tile as tile
from concourse import bacc
from concourse.bass_utils import run_bass_kernel_spmd

NCORES = 8
F32 = mybir.dt.float32
BF16 = mybir.dt.bfloat16
NPBF16 = ml_dtypes.bfloat16


def _build(S, N2, ITEM, DIMP, denom, topk_f):
    """Build the per-core Bass program.

    S     - x1 rows per core (multiple of 128)
    N2    - x2 rows (multiple of 512)
    ITEM  - feature dim (multiple of 512)
    DIMP  - projection dim padded to a multiple of 128
    denom - sqrt(original ITEM)
    """
    IC = S // 128  # own-row chunks
    JC = N2 // 128  # x2-row chunks (spmm contraction, eT partition blocks)
    JN = N2 // 512  # 512-wide blocks of x2 rows (one per core's K shard)
    TC = ITEM // 128  # feature chunks (projection contraction)
    TN = ITEM // 512  # 512-wide tiles of the output free dim
    DC = DIMP // 128  # projection-dim chunks
    DH = DC // 2  # d-chunks per AllGather half
    assert JN == NCORES and S == 512
    Exp = mybir.ActivationFunctionType.Exp
    Mult = mybir.AluOpType.mult
    Add = mybir.AluOpType.add
    X = mybir.AxisListType.X

    nc = bacc.Bacc("TRN2", target_bir_lowering=False, debug=False, num_devices=NCORES)
    x1t = nc.dram_tensor("x1t", [128, TC, S], BF16, kind="ExternalInput")
    wqt = nc.dram_tensor("wqt", [DC, 128, TC, 128], BF16, kind="ExternalInput")
    wkt = nc.dram_tensor("wkt", [DC, 128, TC, 128], BF16, kind="ExternalInput")
    x2m = nc.dram_tensor("x2m", [128, TC, 512], BF16, kind="ExternalInput")
    x2n = nc.dram_tensor("x2n", [TN, 128, JC, 512], BF16, kind="ExternalInput")
    lmt = nc.dram_tensor("lmt", [128, JC, S], BF16, kind="ExternalInput")
    lmn = nc.dram_tensor("lmn", [JN, 128, IC, 512], BF16, kind="ExternalInput")
    bq2 = nc.dram_tensor("bq2", [128, DC], F32, kind="ExternalInput")
    y = nc.dram_tensor("y", [S, ITEM], F32, kind="ExternalOutput")

    with tile.TileContext(nc) as tc:
        with (
            tc.tile_pool(name="big", bufs=1) as big,
            tc.tile_pool(name="persist", bufs=1) as persist,
            tc.tile_pool(name="s8k", bufs=16) as s8k,
            tc.tile_pool(name="lmtp", bufs=8) as lmtp,
            tc.tile_pool(name="lmnp", bufs=2) as lmnp,
            tc.tile_pool(name="outp", bufs=4) as outp,
            tc.tile_pool(name="dram", bufs=1, space="DRAM") as drampool,
            tc.tile_pool(name="acc", bufs=8, space="PSUM") as accp,
        ):
            ones_b = persist.tile([128, 1], BF16, tag="onesb")
            nc.gpsimd.memset(ones_b[:], 1.0)
            ones_f = persist.tile([128, 1], F32, tag="onesf")
            nc.gpsimd.memset(ones_f[:], 1.0)
            bqs = persist.tile([128, DC], F32, tag="bqs")
            nc.sync.dma_start(bqs[:], bq2[:])

            # ---- phase A: own K.T shard projection, chunked AllGather ----
            # DMA queues (only sync/scalar/gpsimd can issue): the first K-proj
            # matmul is gated by wk0's first chunk (gpsimd) + x2m slab 0's
            # first half (scalar), each 512KB, so PE starts ~5us in.  Per-
            # engine emission order is chosen so every stream lands just
            # ahead of its consumer at ~125GB/s per queue.
            wk = []
            for d in range(DC):
                wk.append(s8k.tile([128, TC, 128], BF16, tag="s8", name=f"wk_{d}"))
            nc.gpsimd.dma_start(wk[0][:, 0:16, :], wkt[0, :, 0:16, :])
            nc.gpsimd.dma_start(wk[0][:, 16:32, :], wkt[0, :, 16:32, :])
            xm = []
            for q in range(4):
                xm.append(s8k.tile([128, 8, 512], BF16, tag="s8", name=f"x2m_{q}"))
                nc.scalar.dma_start(xm[q][:, 0:4, :], x2m[:, q * 8 : q * 8 + 4, :])
            for q in range(4):
                nc.sync.dma_start(xm[q][:, 4:8, :], x2m[:, q * 8 + 4 : q * 8 + 8, :])
            # wk0..wk4 + wk6 on gpsimd (the early chains gate the pipeline);
            # wk5/wk7 trail the x2m halves on sync
            for d in (1, 2, 3, 4, 6):
                nc.gpsimd.dma_start(wk[d][:], wkt[d])
            for d in (5, 7):
                nc.sync.dma_start(wk[d][:], wkt[d])
            # x1 for phase 1: q0/q2 on scalar now, q4/q6 after the ktin0 store,
            # odd chunks on sync
            x1s = big.tile([128, TC, S], BF16, tag="bigA", name="x1s")
            for q in (0, 2, 1, 3, 5, 7):
                eng = nc.scalar if q % 2 == 0 else nc.sync
                eng.dma_start(
                    x1s[:, q * 4 : (q + 1) * 4, :], x1t[:, q * 4 : (q + 1) * 4, :]
                )
            ktsb = persist.tile([128, DC, 512], BF16, tag="ktsb")
            # 3-way chunked AllGather: launch after d1 / d4 / d7 so the gather
            # bandwidth overlaps the rest of phase A and the Q projection
            CCS = [(0, 2), (2, 5), (5, 8)]
            ktin = [
                drampool.tile(
                    [128, hi - lo, 512], BF16, tag=f"ktin{h}", name=f"ktin{h}"
                )
                for h, (lo, hi) in enumerate(CCS)
            ]
            ktall = [
                drampool.tile(
                    [NCORES, 128, hi - lo, 512], BF16, tag=f"ktall{h}",
                    name=f"ktall{h}", addr_space="Shared",
                )
                for h, (lo, hi) in enumerate(CCS)
            ]
            for d in range(DC):
                ps = accp.tile([128, 512], F32, tag="acc", name=f"psk_{d}")
                for t in range(TC):
                    nc.tensor.matmul(
                        ps[:],
                        wk[d][:, t, :],
                        xm[t // 8][:, t % 8, :],
                        start=(t == 0),
                        stop=(t == TC - 1),
                    )
                nc.scalar.copy(ktsb[:, d, :], ps[:])
                for h, (lo, hi) in enumerate(CCS):
                    if d == hi - 1:
                        nc.scalar.dma_start(ktin[h][:], ktsb[:, lo:hi, :])
                        nc.gpsimd.collective_compute(
                            "AllGather",
                            mybir.AluOpType.bypass,
                            replica_groups=[list(range(NCORES))],
                            ins=[ktin[h][:].opt()],
                            outs=[ktall[h][:].opt()],
                        )
                if d == 2:
                    for q in (4, 6):
                        nc.scalar.dma_start(
                            x1s[:, q * 4 : (q + 1) * 4, :],
                            x1t[:, q * 4 : (q + 1) * 4, :],
                        )

            # ---- phase 1: QT[d, i] = ((x1 @ Wq.T) + bq) / denom, DIM-major ----
            qt = persist.tile([128, DC, S], BF16, tag="qt")
            for d in range(DC):
                wsl = s8k.tile([128, TC, 128], BF16, tag="s8", name=f"wq_{d}")
                weng = nc.gpsimd if d % 2 == 0 else nc.sync
                weng.dma_start(wsl[:], wqt[d])
                ps = accp.tile([128, 512], F32, tag="acc", name=f"psq_{d}")
                for t in range(TC):
                    nc.tensor.matmul(
                        ps[:],
                        wsl[:, t, :],
                        x1s[:, t, :],
                        start=(t == 0),
                        stop=(t == TC - 1),
                    )
                nc.vector.tensor_scalar(
                    qt[:, d, :], ps[:], 1.0 / denom, bqs[:, d : d + 1],
                    op0=Mult, op1=Add,
                )

            # ---- interactions = rowsum(label), on the idle vector engine ----
            iparts = persist.tile([128, IC, JN], F32, tag="iparts")
            for jn in range(JN):
                lsl = lmnp.tile([128, IC, 512], BF16, tag="lmn")
                nc.gpsimd.dma_start(lsl[:], lmn[jn])
                for i in range(IC):
                    nc.vector.reduce_sum(iparts[:, i, jn : jn + 1], lsl[:, i, :], axis=X)
            ia = persist.tile([128, IC, 1], F32, tag="ia")
            nc.vector.reduce_sum(ia[:], iparts[:], axis=X)
            nc.vector.tensor_scalar(ia[:], ia[:], 1.0 / topk_f, 1e-8, op0=Mult, op1=Add)

            # ---- phase 3: transposed scores -> exp -> eT tiles + e row-sums ----
            et = big.tile([128, JC, S], BF16, tag="bigA", name="et")
            esum = accp.tile([1, 512], F32, tag="acc", name="esum")
            # the e row-sum matmuls lag 2 tiles behind the exp pipeline so the
            # PE never waits on the vector-add + exp epilogue of the same tile
            pend = []

            def flush_esum(limit):
                while len(pend) > limit:
                    j0 = pend.pop(0)
                    nc.tensor.matmul(
                        esum[:], ones_b[:], et[:, j0, :],
                        start=(j0 == 0), stop=(j0 == JC - 1),
                    )

            ch_first = []
            for jn in range(JN):
                lmcs = []
                for jl in range(4):
                    jj = jn * 4 + jl
                    lmc = lmtp.tile([128, 1, 512], BF16, tag="lmt", name=f"lm_{jj}")
                    nc.sync.dma_start(lmc[:], lmt[:, jj : jj + 1, :])
                    lmcs.append(lmc)
                kt = s8k.tile([128, DC, 512], BF16, tag="s8", name=f"kt_{jn}")
                nc.scalar.dma_start(kt[:, 0:2, :], ktall[0][jn])
                nc.gpsimd.dma_start(kt[:, 2:5, :], ktall[1][jn])
                nc.scalar.dma_start(kt[:, 5:8, :], ktall[2][jn])
                if jn == 2:
                    # n=0 spmm chunks: queued behind kt_0..2 so the K.T tiles
                    # win the queue race; they land during phase 3.
                    for jq in range(4):
                        c = s8k.tile(
                            [128, 8, 512], BF16, tag="s8", name=f"x2c_0_{jq}"
                        )
                        eng = nc.scalar if jq % 2 == 0 else nc.gpsimd
                        eng.dma_start(c[:], x2n[0, :, jq * 8 : (jq + 1) * 8, :])
                        ch_first.append(c)
                for jl in range(4):
                    jj = jn * 4 + jl
                    lmc = lmcs[jl]
                    ps = accp.tile([128, 512], F32, tag="acc", name=f"ps3_{jj}")
                    for d in range(DC):
                        nc.tensor.matmul(
                            ps[:],
                            kt[:, d, jl * 128 : (jl + 1) * 128],
                            qt[:, d, :],
                            start=(d == 0),
                            stop=(d == DC - 1),
                        )
                    nc.vector.tensor_add(ps[:], ps[:], lmc[:, 0, :])
                    nc.scalar.activation(et[:, jj, :], ps[:], Exp)
                    pend.append(jj)
                    flush_esum(2)
            flush_esum(0)

            # ---- a_i = (interactions/topk + 1e-8) / (sum_e + 1e-8), column-major ----
            esr = persist.tile([1, 512], F32, tag="esr")
            nc.scalar.copy(esr[:], esum[:])
            ecol = persist.tile([128, IC, 1], F32, tag="ecol")
            for i in range(IC):
                pt = accp.tile([128, 1], F32, tag="acc", name=f"tr_{i}")
                nc.tensor.transpose(
                    pt[:], esr[:, i * 128 : (i + 1) * 128], ones_f[0:1, 0:1]
                )
                nc.scalar.copy(ecol[:, i, :], pt[:])
            rec = persist.tile([128, IC, 1], F32, tag="rec")
            nc.vector.tensor_scalar_add(ecol[:], ecol[:], 1e-8)
            nc.vector.reciprocal(rec[:], ecol[:])
            asb = persist.tile([128, IC, 1], F32, tag="asb")
            nc.vector.tensor_mul(asb[:], ia[:], rec[:])

            # ---- phase 4: spmm, one 32-matmul chain per output tile ----
            for n in range(TN):
                if n == 0:
                    ch = ch_first
                else:
                    ch = []
                    for jq in range(4):
                        c = s8k.tile([128, 8, 512], BF16, tag="s8", name=f"x2c_{n}_{jq}")
                        eng = nc.scalar if jq % 2 == 0 else nc.gpsimd
                        eng.dma_start(c[:], x2n[n, :, jq * 8 : (jq + 1) * 8, :])
                        ch.append(c)
                for i in range(IC):
                    ps = accp.tile([128, 512], F32, tag="acc", name=f"ps4_{n}_{i}")
                    for j in range(JC):
                        nc.tensor.matmul(
                            ps[:],
                            et[:, j, i * 128 : (i + 1) * 128],
                            ch[j // 8][:, j % 8, :],
                            start=(j == 0),
                            stop=(j == JC - 1),
                        )
                    o = outp.tile([128, 512], F32, tag="o")
                    if i % 2 == 0:
                        nc.vector.tensor_scalar_mul(o[:], ps[:], asb[:, i, :])
                    else:
                        nc.scalar.mul(o[:], ps[:], asb[:, i, :])
                    nc.sync.dma_start(
                        y[i * 128 : (i + 1) * 128, n * 512 : (n + 1) * 512], o[:]
                    )

    nc.compile()
    return nc


def _pmajor(a, p, inner):
    """[R, C] with R = nblk*p -> [p, nblk, C] partition-major, where each
    partition's inner block is contiguous."""
    R, C = a.shape
    nblk = R // p
    return np.ascontiguousarray(a.reshape(nblk, p, C).transpose(1, 0, 2))


def _in_maps(x1, x2, label_map, Wq, bq, Wk, DIMP, S, denom):
    ITEM = x1.shape[1]
    N2 = x2.shape[0]
    DIM = Wq.shape[0]
    DC = DIMP // 128
    TC = ITEM // 128
    JN = N2 // 512
    TN = ITEM // 512
    JC = N2 // 128
    IC = S // 128

    wqp = np.zeros((DIMP, ITEM), NPBF16)
    wqp[:DIM] = Wq.astype(NPBF16)
    wkp = np.zeros((DIMP, ITEM), NPBF16)
    wkp[:DIM] = Wk.astype(NPBF16)
    bqp = np.zeros((DIMP,), np.float32)
    bqp[:DIM] = bq / denom
    bq2 = np.ascontiguousarray(bqp.reshape(DC, 128).T)

    x1b = x1.astype(NPBF16)
    x2b = x2.astype(NPBF16)
    wqT = np.ascontiguousarray(wqp.T)  # [ITEM, DIMP]
    x2T = np.ascontiguousarray(x2b.T)  # [ITEM, N2]

    # wqt[d] = WqT[:, d-chunk] as [128, TC, 128] partition-major
    wqt = np.stack(
        [_pmajor(wqT[:, d * 128 : (d + 1) * 128], 128, None) for d in range(DC)]
    )
    wkT = np.ascontiguousarray(wkp.T)
    wktb = np.stack(
        [_pmajor(wkT[:, d * 128 : (d + 1) * 128], 128, None) for d in range(DC)]
    )
    # x2t[jn] = x2T[:, jn-chunk] as [128, TC, 512]
    x2tb = np.stack(
        [_pmajor(x2T[:, j * 512 : (j + 1) * 512], 128, None) for j in range(JN)]
    )
    # x2n[n] = x2[:, n-chunk] as [128, JC, 512]
    x2nb = np.stack(
        [_pmajor(x2b[:, n * 512 : (n + 1) * 512], 128, None) for n in range(TN)]
    )
    maps = []
    for c in range(NCORES):
        sl = slice(c * S, (c + 1) * S)
        shard = label_map[sl]
        # normal orientation (0/1) for interaction row sums
        lmb = np.stack(
            [
                _pmajor(shard.astype(NPBF16)[:, j * 512 : (j + 1) * 512], 128, None)
                for j in range(JN)
            ]
        )
        # transposed additive mask: 0 where label=1, -30 where label=0
        mt = ((shard.T.astype(np.float32) - 1.0) * 30.0).astype(NPBF16)  # [N2, S]
        lmtb = _pmajor(mt, 128, None)  # [128, JC, S]
        maps.append(
            {
                "x1t": _pmajor(np.ascontiguousarray(x1b[sl].T), 128, None),
                "wqt": wqt,
                "wkt": wktb,
                "x2m": x2tb[c],
                "x2n": x2nb,
                "lmt": lmtb,
                "lmn": lmb,
                "bq2": bq2,
            }
        )
    return maps


def _run(x1, x2, label_map, Wq, bq, Wk, bk, topk, trace=False):
    x1 = np.asarray(x1, np.float32)
    x2 = np.asarray(x2, np.float32)
    label_map = np.asarray(label_map, np.float32)
    Wq = np.asarray(Wq, np.float32)
    bq = np.asarray(bq, np.float32)
    Wk = np.asarray(Wk, np.float32)
    N1, ITEM = x1.shape
    N2 = x2.shape[0]
    DIM = Wq.shape[0]
    S = N1 // NCORES
    DIMP = ((DIM + 127) // 128) * 128
    denom = math.sqrt(ITEM)
    nc = _build(S, N2, ITEM, DIMP, denom, float(topk))
    maps = _in_maps(x1, x2, label_map, Wq, bq, Wk, DIMP, S, denom)
    res = run_bass_kernel_spmd(
        nc, maps, list(range(NCORES)), trace=trace, trace_cores=[0] if trace else None
    )
    out = np.concatenate([res.results[c]["y"] for c in range(NCORES)], axis=0)
    return out.astype(np.float32), res


def kernel(x1, x2, label_map, Wq, bq, Wk, bk, topk):
    out, _ = _run(x1, x2, label_map, Wq, bq, Wk, bk, topk)
    return out


# revision 15
# speedup vs baseline: 1.1539x; 1.0137x over previous
"""Sparse-attention kernel for 8 trn2 NeuronCores (Bass/Tile) — v6.

Math (reference):
    Q = x1 @ Wq.T + bq                       [N1, DIM]
    K = x2 @ Wk.T + bk                       [N2, DIM]
    scores = (Q @ K.T) / sqrt(ITEM)          [N1, N2]
    e = exp(scores) * label_map
    att = e / (sum_j e + 1e-8) * (sum_j label_map / topk + 1e-8)
    out = att @ x2                           [N1, ITEM]

Key transformations (on top of the v5 baseline):
  * Rows of x1/label_map sharded across 8 cores (512 rows each); bk drops
    out of the normalization (scales numerator and denominator equally).
  * Each core projects only its own 512-column shard of K.T; the shard is
    AllGathered in TWO d-halves so the first collective launches mid-way
    through the K projection and both finish under the Q projection.
  * Scores are computed TRANSPOSED (eT tiles [n2-rows, own-rows]) by
    swapping matmul operands: weights = K.T chunks, moving = Q.T rows.
    This removes all 128 PE transposes + 128 scalar copies of v5: the exp
    output lands directly in the spmm operand layout.
  * label masking is folded into the exp argument: host ships
    M = (label-1)*30 and the kernel computes e = exp(scores + M), so the
    masked entries underflow to ~e-26 (negligible vs sums ~1e3).
  * Row sums of e (the softmax denominator) use a ones-weight matmul chain
    accumulated across all 32 eT tiles; interactions = rowsum(label) is
    reduced on the (otherwise idle) vector engine from a second,
    untransposed copy of the label map during the Q projection.
  * spmm processes output columns in PAIRS: each et weight tile is loaded
    once per 2 matmuls (1024 PE cycles), halving LDWEIGHTS traffic; the 8
    PSUM banks hold 4 row-chunks x 2 column-tiles of accumulators, and
    drains are split across the vector and scalar engines.
  * DMA streams are spread over 4 queues (sync/scalar/vector/gpsimd) and
    issued in ~256-512KB pieces so the first K-proj matmul starts ~5us in
    and no phase waits on a single-queue stream (~125GB/s per queue).
  * Matmul operands are bf16 (fp32 PSUM accumulation); weights partition-
    major so every SBUF slab loads with fully contiguous multi-KB lines.
"""

import math

import numpy as np

try:
    import concourse.bass as bass
except ImportError:  # fresh interpreter without the boot path
    import sys

    sys.path.insert(0, "/opt/trn_rl_repo")
    import concourse.bass as bass

import ml_dtypes
import concourse.mybir as mybir
import concourse.tile as tile
from concourse import bacc
from concourse.bass_utils import run_bass_kernel_spmd

NCORES = 8
F32 = mybir.dt.float32
BF16 = mybir.dt.bfloat16
NPBF16 = ml_dtypes.bfloat16


def _build(S, N2, ITEM, DIMP, denom, topk_f):
    """Build the per-core Bass program.

    S     - x1 rows per core (multiple of 128)
    N2    - x2 rows (multiple of 512)
    ITEM  - feature dim (multiple of 512)
    DIMP  - projection dim padded to a multiple of 128
    denom - sqrt(original ITEM)
    """
    IC = S // 128  # own-row chunks
    JC = N2 // 128  # x2-row chunks (spmm contraction, eT partition blocks)
    JN = N2 // 512  # 512-wide blocks of x2 rows (one per core's K shard)
    TC = ITEM // 128  # feature chunks (projection contraction)
    TN = ITEM // 512  # 512-wide tiles of the output free dim
    DC = DIMP // 128  # projection-dim chunks
    DH = DC // 2  # d-chunks per AllGather half
    assert JN == NCORES and S == 512
    Exp = mybir.ActivationFunctionType.Exp
    Mult = mybir.AluOpType.mult
    Add = mybir.AluOpType.add
    X = mybir.AxisListType.X

    nc = bacc.Bacc("TRN2", target_bir_lowering=False, debug=False, num_devices=NCORES)
    x1t = nc.dram_tensor("x1t", [128, TC, S], BF16, kind="ExternalInput")
    wqt = nc.dram_tensor("wqt", [DC, 128, TC, 128], BF16, kind="ExternalInput")
    wkt = nc.dram_tensor("wkt", [DC, 128, TC, 128], BF16, kind="ExternalInput")
    x2m = nc.dram_tensor("x2m", [128, TC, 512], BF16, kind="ExternalInput")
    x2n = nc.dram_tensor("x2n", [TN, 128, JC, 512], BF16, kind="ExternalInput")
    lmt = nc.dram_tensor("lmt", [128, JC, S], BF16, kind="ExternalInput")
    lmn = nc.dram_tensor("lmn", [JN, 128, IC, 512], BF16, kind="ExternalInput")
    bq2 = nc.dram_tensor("bq2", [128, DC], F32, kind="ExternalInput")
    y = nc.dram_tensor("y", [S, ITEM], F32, kind="ExternalOutput")

    with tile.TileContext(nc) as tc:
        with (
            tc.tile_pool(name="big", bufs=1) as big,
            tc.tile_pool(name="persist", bufs=1) as persist,
            tc.tile_pool(name="s8k", bufs=16) as s8k,
            tc.tile_pool(name="lmtp", bufs=8) as lmtp,
            tc.tile_pool(name="lmnp", bufs=2) as lmnp,
            tc.tile_pool(name="outp", bufs=4) as outp,
            tc.tile_pool(name="dram", bufs=1, space="DRAM") as drampool,
            tc.tile_pool(name="acc", bufs=8, space="PSUM") as accp,
        ):
            ones_b = persist.tile([128, 1], BF16, tag="onesb")
            nc.gpsimd.memset(ones_b[:], 1.0)
            ones_f = persist.tile([128, 1], F32, tag="onesf")
            nc.gpsimd.memset(ones_f[:], 1.0)
            bqs = persist.tile([128, DC], F32, tag="bqs")
            nc.sync.dma_start(bqs[:], bq2[:])

            # ---- phase A: own K.T shard projection, chunked AllGather ----
            # DMA queues (only sync/scalar/gpsimd can issue): the first K-proj
            # matmul is gated by wk0's first chunk (gpsimd) + x2m slab 0's
            # first half (scalar), each 512KB, so PE starts ~5us in.  Per-
            # engine emission order is chosen so every stream lands just
            # ahead of its consumer at ~125GB/s per queue.
            wk = []
            for d in range(DC):
                wk.append(s8k.tile([128, TC, 128], BF16, tag="s8", name=f"wk_{d}"))
            nc.gpsimd.dma_start(wk[0][:, 0:16, :], wkt[0, :, 0:16, :])
            nc.gpsimd.dma_start(wk[0][:, 16:32, :], wkt[0, :, 16:32, :])
            xm = []
            for q in range(4):
                xm.append(s8k.tile([128, 8, 512], BF16, tag="s8", name=f"x2m_{q}"))
                nc.scalar.dma_start(xm[q][:, 0:4, :], x2m[:, q * 8 : q * 8 + 4, :])
            for q in range(4):
                nc.sync.dma_start(xm[q][:, 4:8, :], x2m[:, q * 8 + 4 : q * 8 + 8, :])
            # wk0..wk4 + wk6 on gpsimd (the early chains gate the pipeline);
            # wk5/wk7 trail the x2m halves on sync
            for d in (1, 2, 3, 4, 6):
                nc.gpsimd.dma_start(wk[d][:], wkt[d])
            for d in (5, 7):
                nc.sync.dma_start(wk[d][:], wkt[d])
            # x1 for phase 1: q0/q2 on scalar now, q4/q6 after the ktin0 store,
            # odd chunks on sync
            x1s = big.tile([128, TC, S], BF16, tag="bigA", name="x1s")
            for q in (0, 2, 1, 3, 5, 7):
                eng = nc.scalar if q % 2 == 0 else nc.sync
                eng.dma_start(
                    x1s[:, q * 4 : (q + 1) * 4, :], x1t[:, q * 4 : (q + 1) * 4, :]
                )
            ktsb = persist.tile([128, DC, 512], BF16, tag="ktsb")
            # 3-way chunked AllGather: launch after d1 / d4 / d7 so the gather
            # bandwidth overlaps the rest of phase A and the Q projection
            CCS = [(0, 3), (3, 6), (6, 8)]
            ktin = [
                drampool.tile(
                    [128, hi - lo, 512], BF16, tag=f"ktin{h}", name=f"ktin{h}"
                )
                for h, (lo, hi) in enumerate(CCS)
            ]
            ktall = [
                drampool.tile(
                    [NCORES, 128, hi - lo, 512], BF16, tag=f"ktall{h}",
                    name=f"ktall{h}", addr_space="Shared",
                )
                for h, (lo, hi) in enumerate(CCS)
            ]
            for d in range(DC):
                ps = accp.tile([128, 512], F32, tag="acc", name=f"psk_{d}")
                for t in range(TC):
                    nc.tensor.matmul(
                        ps[:],
                        wk[d][:, t, :],
                        xm[t // 8][:, t % 8, :],
                        start=(t == 0),
                        stop=(t == TC - 1),
                    )
                nc.vector.tensor_copy(ktsb[:, d, :], ps[:])
                for h, (lo, hi) in enumerate(CCS):
                    if d == hi - 1:
                        nc.scalar.dma_start(ktin[h][:], ktsb[:, lo:hi, :])
                        nc.gpsimd.collective_compute(
                            "AllGather",
                            mybir.AluOpType.bypass,
                            replica_groups=[list(range(NCORES))],
                            ins=[ktin[h][:].opt()],
                            outs=[ktall[h][:].opt()],
                        )
                if d == 2:
                    for q in (4, 6):
                        nc.scalar.dma_start(
                            x1s[:, q * 4 : (q + 1) * 4, :],
                            x1t[:, q * 4 : (q + 1) * 4, :],
                        )

            # ---- phase 1: QT[d, i] = ((x1 @ Wq.T) + bq) / denom, DIM-major ----
            qt = persist.tile([128, DC, S], BF16, tag="qt")
            for d in range(DC):
                wsl = s8k.tile([128, TC, 128], BF16, tag="s8", name=f"wq_{d}")
                weng = nc.gpsimd if d % 2 == 0 else nc.sync
                weng.dma_start(wsl[:], wqt[d])
                ps = accp.tile([128, 512], F32, tag="acc", name=f"psq_{d}")
                for t in range(TC):
                    nc.tensor.matmul(
                        ps[:],
                        wsl[:, t, :],
                        x1s[:, t, :],
                        start=(t == 0),
                        stop=(t == TC - 1),
                    )
                nc.vector.tensor_scalar(
                    qt[:, d, :], ps[:], 1.0 / denom, bqs[:, d : d + 1],
                    op0=Mult, op1=Add,
                )

            # ---- interactions = rowsum(label), on the idle vector engine ----
            iparts = persist.tile([128, IC, JN], F32, tag="iparts")
            for jn in range(JN):
                lsl = lmnp.tile([128, IC, 512], BF16, tag="lmn")
                nc.gpsimd.dma_start(lsl[:], lmn[jn])
                for i in range(IC):
                    nc.vector.reduce_sum(iparts[:, i, jn : jn + 1], lsl[:, i, :], axis=X)
            ia = persist.tile([128, IC, 1], F32, tag="ia")
            nc.vector.reduce_sum(ia[:], iparts[:], axis=X)
            nc.vector.tensor_scalar(ia[:], ia[:], 1.0 / topk_f, 1e-8, op0=Mult, op1=Add)

            # ---- phase 3: transposed scores -> exp -> eT tiles + e row-sums ----
            et = big.tile([128, JC, S], BF16, tag="bigA", name="et")
            esum = accp.tile([1, 512], F32, tag="acc", name="esum")
            # the e row-sum matmuls lag 2 tiles behind the exp pipeline so the
            # PE never waits on the vector-add + exp epilogue of the same tile
            pend = []

            def flush_esum(limit):
                while len(pend) > limit:
                    j0 = pend.pop(0)
                    nc.tensor.matmul(
                        esum[:], ones_b[:], et[:, j0, :],
                        start=(j0 == 0), stop=(j0 == JC - 1),
                    )

            ch_first = []
            for jn in range(JN):
                lmcs = []
                for jl in range(4):
                    jj = jn * 4 + jl
                    lmc = lmtp.tile([128, 1, 512], BF16, tag="lmt", name=f"lm_{jj}")
                    nc.sync.dma_start(lmc[:], lmt[:, jj : jj + 1, :])
                    lmcs.append(lmc)
                kt = s8k.tile([128, DC, 512], BF16, tag="s8", name=f"kt_{jn}")
                nc.scalar.dma_start(kt[:, 0:3, :], ktall[0][jn])
                nc.gpsimd.dma_start(kt[:, 3:6, :], ktall[1][jn])
                nc.scalar.dma_start(kt[:, 6:8, :], ktall[2][jn])
                if jn == 2:
                    # n=0 spmm chunks: queued behind kt_0..2 so the K.T tiles
                    # win the queue race; they land during phase 3.
                    for jq in range(4):
                        c = s8k.tile(
                            [128, 8, 512], BF16, tag="s8", name=f"x2c_0_{jq}"
                        )
                        eng = nc.scalar if jq % 2 == 0 else nc.gpsimd
                        eng.dma_start(c[:], x2n[0, :, jq * 8 : (jq + 1) * 8, :])
                        ch_first.append(c)
                for jl in range(4):
                    jj = jn * 4 + jl
                    lmc = lmcs[jl]
                    ps = accp.tile([128, 512], F32, tag="acc", name=f"ps3_{jj}")
                    for d in range(DC):
                        nc.tensor.matmul(
                            ps[:],
                            kt[:, d, jl * 128 : (jl + 1) * 128],
                            qt[:, d, :],
                            start=(d == 0),
                            stop=(d == DC - 1),
                        )
                    nc.vector.tensor_add(ps[:], ps[:], lmc[:, 0, :])
                    nc.scalar.activation(et[:, jj, :], ps[:], Exp)
                    pend.append(jj)
                    flush_esum(2)
            flush_esum(0)

            # ---- a_i = (interactions/topk + 1e-8) / (sum_e + 1e-8), column-major ----
            esr = persist.tile([1, 512], F32, tag="esr")
            nc.scalar.copy(esr[:], esum[:])
            ecol = persist.tile([128, IC, 1], F32, tag="ecol")
            for i in range(IC):
                pt = accp.tile([128, 1], F32, tag="acc", name=f"tr_{i}")
                nc.tensor.transpose(
                    pt[:], esr[:, i * 128 : (i + 1) * 128], ones_f[0:1, 0:1]
                )
                nc.scalar.copy(ecol[:, i, :], pt[:])
            rec = persist.tile([128, IC, 1], F32, tag="rec")
            nc.vector.tensor_scalar_add(ecol[:], ecol[:], 1e-8)
            nc.vector.reciprocal(rec[:], ecol[:])
            asb = persist.tile([128, IC, 1], F32, tag="asb")
            nc.vector.tensor_mul(asb[:], ia[:], rec[:])

            # ---- phase 4: spmm, one 32-matmul chain per output tile ----
            for n in range(TN):
                if n == 0:
                    ch = ch_first
                else:
                    ch = []
                    for jq in range(4):
                        c = s8k.tile([128, 8, 512], BF16, tag="s8", name=f"x2c_{n}_{jq}")
                        eng = nc.scalar if jq % 2 == 0 else nc.gpsimd
                        eng.dma_start(c[:], x2n[n, :, jq * 8 : (jq + 1) * 8, :])
                        ch.append(c)
                for i in range(IC):
                    ps = accp.tile([128, 512], F32, tag="acc", name=f"ps4_{n}_{i}")
                    for j in range(JC):
                        nc.tensor.matmul(
                            ps[:],
                            et[:, j, i * 128 : (i + 1) * 128],
                            ch[j // 8][:, j % 8, :],
                            start=(j == 0),
                            stop=(j == JC - 1),
                        )
                    o = outp.tile([128, 512], F32, tag="o")
                    if i % 2 == 0:
                        nc.vector.tensor_scalar_mul(o[:], ps[:], asb[:, i, :])
                    else:
                        nc.scalar.mul(o[:], ps[:], asb[:, i, :])
                    nc.sync.dma_start(
                        y[i * 128 : (i + 1) * 128, n * 512 : (n + 1) * 512], o[:]
                    )

    nc.compile()
    return nc


def _pmajor(a, p, inner):
    """[R, C] with R = nblk*p -> [p, nblk, C] partition-major, where each
    partition's inner block is contiguous."""
    R, C = a.shape
    nblk = R // p
    return np.ascontiguousarray(a.reshape(nblk, p, C).transpose(1, 0, 2))


def _in_maps(x1, x2, label_map, Wq, bq, Wk, DIMP, S, denom):
    ITEM = x1.shape[1]
    N2 = x2.shape[0]
    DIM = Wq.shape[0]
    DC = DIMP // 128
    TC = ITEM // 128
    JN = N2 // 512
    TN = ITEM // 512
    JC = N2 // 128
    IC = S // 128

    wqp = np.zeros((DIMP, ITEM), NPBF16)
    wqp[:DIM] = Wq.astype(NPBF16)
    wkp = np.zeros((DIMP, ITEM), NPBF16)
    wkp[:DIM] = Wk.astype(NPBF16)
    bqp = np.zeros((DIMP,), np.float32)
    bqp[:DIM] = bq / denom
    bq2 = np.ascontiguousarray(bqp.reshape(DC, 128).T)

    x1b = x1.astype(NPBF16)
    x2b = x2.astype(NPBF16)
    wqT = np.ascontiguousarray(wqp.T)  # [ITEM, DIMP]
    x2T = np.ascontiguousarray(x2b.T)  # [ITEM, N2]

    # wqt[d] = WqT[:, d-chunk] as [128, TC, 128] partition-major
    wqt = np.stack(
        [_pmajor(wqT[:, d * 128 : (d + 1) * 128], 128, None) for d in range(DC)]
    )
    wkT = np.ascontiguousarray(wkp.T)
    wktb = np.stack(
        [_pmajor(wkT[:, d * 128 : (d + 1) * 128], 128, None) for d in range(DC)]
    )
    # x2t[jn] = x2T[:, jn-chunk] as [128, TC, 512]
    x2tb = np.stack(
        [_pmajor(x2T[:, j * 512 : (j + 1) * 512], 128, None) for j in range(JN)]
    )
    # x2n[n] = x2[:, n-chunk] as [128, JC, 512]
    x2nb = np.stack(
        [_pmajor(x2b[:, n * 512 : (n + 1) * 512], 128, None) for n in range(TN)]
    )
    maps = []
    for c in range(NCORES):
        sl = slice(c * S, (c + 1) * S)
        shard = label_map[sl]
        # normal orientation (0/1) for interaction row sums
        lmb = np.stack(
            [
                _pmajor(shard.astype(NPBF16)[:, j * 512 : (j + 1) * 512], 128, None)
                for j in range(JN)
            ]
        )
        # transposed additive mask: 0 where label=1, -30 where label=0
        mt = ((shard.T.astype(np.float32) - 1.0) * 30.0).astype(NPBF16)  # [N2, S]
        lmtb = _pmajor(mt, 128, None)  # [128, JC, S]
        maps.append(
            {
                "x1t": _pmajor(np.ascontiguousarray(x1b[sl].T), 128, None),
                "wqt": wqt,
                "wkt": wktb,
                "x2m": x2tb[c],
                "x2n": x2nb,
                "lmt": lmtb,
                "lmn": lmb,
                "bq2": bq2,
            }
        )
    return maps


def _run(x1, x2, label_map, Wq, bq, Wk, bk, topk, trace=False):
    x1 = np.asarray(x1, np.float32)
    x2 = np.asarray(x2, np.float32)
    label_map = np.asarray(label_map, np.float32)
    Wq = np.asarray(Wq, np.float32)
    bq = np.asarray(bq, np.float32)
    Wk = np.asarray(Wk, np.float32)
    N1, ITEM = x1.shape
    N2 = x2.shape[0]
    DIM = Wq.shape[0]
    S = N1 // NCORES
    DIMP = ((DIM + 127) // 128) * 128
    denom = math.sqrt(ITEM)
    nc = _build(S, N2, ITEM, DIMP, denom, float(topk))
    maps = _in_maps(x1, x2, label_map, Wq, bq, Wk, DIMP, S, denom)
    res = run_bass_kernel_spmd(
        nc, maps, list(range(NCORES)), trace=trace, trace_cores=[0] if trace else None
    )
    out = np.concatenate([res.results[c]["y"] for c in range(NCORES)], axis=0)
    return out.astype(np.float32), res


def kernel(x1, x2, label_map, Wq, bq, Wk, bk, topk):
    out, _ = _run(x1, x2, label_map, Wq, bq, Wk, bk, topk)
    return out


# revision 16
# speedup vs baseline: 1.1602x; 1.0055x over previous
"""Sparse-attention kernel for 8 trn2 NeuronCores (Bass/Tile).

Math (reference):
    Q = x1 @ Wq.T + bq                       [N1, DIM]
    K = x2 @ Wk.T + bk                       [N2, DIM]
    scores = (Q @ K.T) / sqrt(ITEM)          [N1, N2]
    e = exp(scores) * label_map
    att = e / (sum_j e + 1e-8) * (sum_j label_map / topk + 1e-8)
    out = att @ x2                           [N1, ITEM]

Key transformations:
  * Rows of x1/label_map sharded across 8 cores (512 rows each); bk drops
    out of the normalization (it scales numerator and denominator equally).
  * Each core projects only its own 512-column shard of K.T; the shard is
    AllGathered in THREE d-chunks (3/3/2), each launched as soon as its
    d-range of the K projection finishes, so the gather bandwidth overlaps
    the rest of phase A and the Q projection and the (small) last chunk
    completes before the scores phase needs it.
  * Scores are computed TRANSPOSED (eT tiles [n2-rows, own-rows]) by
    swapping matmul operands: weights = K.T chunks, moving = Q.T rows.
    This removes all 128 PE transposes + 128 scalar PSUM copies the
    untransposed form needs: the exp output lands directly in the spmm
    operand layout.
  * label masking is folded into the exp argument: host ships
    M = (label-1)*30 and the kernel computes e = exp(scores + M), so
    masked entries underflow to ~e-26 (negligible vs row sums ~1e3).
  * Row sums of e (the softmax denominator) use a ones-weight matmul
    chain accumulated across all 32 eT tiles, emitted 2 tiles behind the
    exp pipeline so the PE never waits on the vector/scalar epilogue;
    interactions = rowsum(label) is reduced on the otherwise-idle vector
    engine from a second, untransposed copy of the label map.
  * The per-row scale a_i is applied in the spmm drains (split across the
    vector and scalar engines); the row-vector of e-sums is transposed to
    per-partition layout with 4 tiny PE transposes.
  * Only sync/scalar/gpsimd can issue DMAs.  Streams are placed so no
    queue that must wait on a collective output carries anything needed
    earlier, the first K-proj matmul is gated by ~1MB, and every weight/
    activation slab lands just ahead of its consumer (~100GB/s/queue).
  * Matmul operands are bf16 (fp32 PSUM accumulation); everything is
    host-rearranged partition-major so slabs load as contiguous multi-KB
    per-partition lines.
"""

import math

import numpy as np

try:
    import concourse.bass as bass
except ImportError:  # fresh interpreter without the boot path
    import sys

    sys.path.insert(0, "/opt/trn_rl_repo")
    import concourse.bass as bass

import ml_dtypes
import concourse.mybir as mybir
import concourse.tile as tile
from concourse import bacc
from concourse.bass_utils import run_bass_kernel_spmd

NCORES = 8
F32 = mybir.dt.float32
BF16 = mybir.dt.bfloat16
NPBF16 = ml_dtypes.bfloat16


def _build(S, N2, ITEM, DIMP, denom, topk_f):
    """Build the per-core Bass program.

    S     - x1 rows per core (multiple of 128)
    N2    - x2 rows (multiple of 512)
    ITEM  - feature dim (multiple of 512)
    DIMP  - projection dim padded to a multiple of 128
    denom - sqrt(original ITEM)
    """
    IC = S // 128  # own-row chunks
    JC = N2 // 128  # x2-row chunks (spmm contraction, eT partition blocks)
    JN = N2 // 512  # 512-wide blocks of x2 rows (one per core's K shard)
    TC = ITEM // 128  # feature chunks (projection contraction)
    TN = ITEM // 512  # 512-wide tiles of the output free dim
    DC = DIMP // 128  # projection-dim chunks
    DH = DC // 2  # d-chunks per AllGather half
    assert JN == NCORES and S == 512
    Exp = mybir.ActivationFunctionType.Exp
    Mult = mybir.AluOpType.mult
    Add = mybir.AluOpType.add
    X = mybir.AxisListType.X

    nc = bacc.Bacc("TRN2", target_bir_lowering=False, debug=False, num_devices=NCORES)
    x1t = nc.dram_tensor("x1t", [128, TC, S], BF16, kind="ExternalInput")
    wqt = nc.dram_tensor("wqt", [DC, 128, TC, 128], BF16, kind="ExternalInput")
    wkt = nc.dram_tensor("wkt", [DC, 128, TC, 128], BF16, kind="ExternalInput")
    x2m = nc.dram_tensor("x2m", [128, TC, 512], BF16, kind="ExternalInput")
    x2n = nc.dram_tensor("x2n", [TN, 128, JC, 512], BF16, kind="ExternalInput")
    lmt = nc.dram_tensor("lmt", [128, JC, S], BF16, kind="ExternalInput")
    lmn = nc.dram_tensor("lmn", [JN, 128, IC, 512], BF16, kind="ExternalInput")
    bq2 = nc.dram_tensor("bq2", [128, DC], F32, kind="ExternalInput")
    y = nc.dram_tensor("y", [S, ITEM], F32, kind="ExternalOutput")

    with tile.TileContext(nc) as tc:
        with (
            tc.tile_pool(name="big", bufs=1) as big,
            tc.tile_pool(name="persist", bufs=1) as persist,
            tc.tile_pool(name="s8k", bufs=16) as s8k,
            tc.tile_pool(name="lmtp", bufs=8) as lmtp,
            tc.tile_pool(name="lmnp", bufs=2) as lmnp,
            tc.tile_pool(name="outp", bufs=4) as outp,
            tc.tile_pool(name="dram", bufs=1, space="DRAM") as drampool,
            tc.tile_pool(name="acc", bufs=8, space="PSUM") as accp,
        ):
            ones_b = persist.tile([128, 1], BF16, tag="onesb")
            nc.gpsimd.memset(ones_b[:], 1.0)
            ones_f = persist.tile([128, 1], F32, tag="onesf")
            nc.gpsimd.memset(ones_f[:], 1.0)
            bqs = persist.tile([128, DC], F32, tag="bqs")
            nc.sync.dma_start(bqs[:], bq2[:])

            # ---- phase A: own K.T shard projection, chunked AllGather ----
            # DMA queues (only sync/scalar/gpsimd can issue): the first K-proj
            # matmul is gated by wk0's first chunk (gpsimd) + x2m slab 0's
            # first half (scalar), each 512KB, so PE starts ~5us in.  Per-
            # engine emission order is chosen so every stream lands just
            # ahead of its consumer at ~125GB/s per queue.
            wk = []
            for d in range(DC):
                wk.append(s8k.tile([128, TC, 128], BF16, tag="s8", name=f"wk_{d}"))
            nc.gpsimd.dma_start(wk[0][:, 0:16, :], wkt[0, :, 0:16, :])
            nc.gpsimd.dma_start(wk[0][:, 16:32, :], wkt[0, :, 16:32, :])
            xm = []
            for q in range(4):
                xm.append(s8k.tile([128, 8, 512], BF16, tag="s8", name=f"x2m_{q}"))
                nc.scalar.dma_start(xm[q][:, 0:4, :], x2m[:, q * 8 : q * 8 + 4, :])
            for q in range(4):
                nc.sync.dma_start(xm[q][:, 4:8, :], x2m[:, q * 8 + 4 : q * 8 + 8, :])
            # wk0..wk4 + wk6 on gpsimd (the early chains gate the pipeline);
            # wk5/wk7 trail the x2m halves on sync
            for d in (1, 2, 3, 4, 6):
                nc.gpsimd.dma_start(wk[d][:], wkt[d])
            for d in (5, 7):
                nc.sync.dma_start(wk[d][:], wkt[d])
            # x1 for phase 1: q0/q2 on scalar now, q4/q6 after the ktin0 store,
            # odd chunks on sync
            x1s = big.tile([128, TC, S], BF16, tag="bigA", name="x1s")
            for q in (0, 2, 1, 3, 5, 7):
                eng = nc.scalar if q % 2 == 0 else nc.sync
                eng.dma_start(
                    x1s[:, q * 4 : (q + 1) * 4, :], x1t[:, q * 4 : (q + 1) * 4, :]
                )
            ktsb = persist.tile([128, DC, 512], BF16, tag="ktsb")
            # 3-way chunked AllGather: launch after d1 / d4 / d7 so the gather
            # bandwidth overlaps the rest of phase A and the Q projection
            CCS = [(0, 3), (3, 6), (6, 8)]
            ktin = [
                drampool.tile(
                    [128, hi - lo, 512], BF16, tag=f"ktin{h}", name=f"ktin{h}"
                )
                for h, (lo, hi) in enumerate(CCS)
            ]
            ktall = [
                drampool.tile(
                    [NCORES, 128, hi - lo, 512], BF16, tag=f"ktall{h}",
                    name=f"ktall{h}", addr_space="Shared",
                )
                for h, (lo, hi) in enumerate(CCS)
            ]
            for d in range(DC):
                ps = accp.tile([128, 512], F32, tag="acc", name=f"psk_{d}")
                for t in range(TC):
                    nc.tensor.matmul(
                        ps[:],
                        wk[d][:, t, :],
                        xm[t // 8][:, t % 8, :],
                        start=(t == 0),
                        stop=(t == TC - 1),
                    )
                nc.vector.tensor_copy(ktsb[:, d, :], ps[:])
                for h, (lo, hi) in enumerate(CCS):
                    if d == hi - 1:
                        nc.scalar.dma_start(ktin[h][:], ktsb[:, lo:hi, :])
                        nc.gpsimd.collective_compute(
                            "AllGather",
                            mybir.AluOpType.bypass,
                            replica_groups=[list(range(NCORES))],
                            ins=[ktin[h][:].opt()],
                            outs=[ktall[h][:].opt()],
                        )
                if d == 2:
                    for q in (4, 6):
                        nc.scalar.dma_start(
                            x1s[:, q * 4 : (q + 1) * 4, :],
                            x1t[:, q * 4 : (q + 1) * 4, :],
                        )

            # ---- phase 1: QT[d, i] = ((x1 @ Wq.T) + bq) / denom, DIM-major ----
            qt = persist.tile([128, DC, S], BF16, tag="qt")
            for d in range(DC):
                wsl = s8k.tile([128, TC, 128], BF16, tag="s8", name=f"wq_{d}")
                weng = nc.gpsimd if d % 2 == 0 else nc.sync
                weng.dma_start(wsl[:], wqt[d])
                ps = accp.tile([128, 512], F32, tag="acc", name=f"psq_{d}")
                for t in range(TC):
                    nc.tensor.matmul(
                        ps[:],
                        wsl[:, t, :],
                        x1s[:, t, :],
                        start=(t == 0),
                        stop=(t == TC - 1),
                    )
                nc.vector.tensor_scalar(
                    qt[:, d, :], ps[:], 1.0 / denom, bqs[:, d : d + 1],
                    op0=Mult, op1=Add,
                )

            # ---- interactions = rowsum(label), on the idle vector engine ----
            iparts = persist.tile([128, IC, JN], F32, tag="iparts")
            for jn in range(JN):
                lsl = lmnp.tile([128, IC, 512], BF16, tag="lmn")
                nc.gpsimd.dma_start(lsl[:], lmn[jn])
                for i in range(IC):
                    nc.vector.reduce_sum(iparts[:, i, jn : jn + 1], lsl[:, i, :], axis=X)
            ia = persist.tile([128, IC, 1], F32, tag="ia")
            nc.vector.reduce_sum(ia[:], iparts[:], axis=X)
            nc.vector.tensor_scalar(ia[:], ia[:], 1.0 / topk_f, 1e-8, op0=Mult, op1=Add)

            # ---- phase 3: transposed scores -> exp -> eT tiles + e row-sums ----
            et = big.tile([128, JC, S], BF16, tag="bigA", name="et")
            esum = accp.tile([1, 512], F32, tag="acc", name="esum")
            # the e row-sum matmuls lag 2 tiles behind the exp pipeline so the
            # PE never waits on the vector-add + exp epilogue of the same tile
            pend = []

            def flush_esum(limit):
                while len(pend) > limit:
                    j0 = pend.pop(0)
                    nc.tensor.matmul(
                        esum[:], ones_b[:], et[:, j0, :],
                        start=(j0 == 0), stop=(j0 == JC - 1),
                    )

            ch_first = []
            for jn in range(JN):
                lmcs = []
                for jl in range(4):
                    jj = jn * 4 + jl
                    lmc = lmtp.tile([128, 1, 512], BF16, tag="lmt", name=f"lm_{jj}")
                    nc.sync.dma_start(lmc[:], lmt[:, jj : jj + 1, :])
                    lmcs.append(lmc)
                kt = s8k.tile([128, DC, 512], BF16, tag="s8", name=f"kt_{jn}")
                nc.scalar.dma_start(kt[:, 0:3, :], ktall[0][jn])
                nc.gpsimd.dma_start(kt[:, 3:6, :], ktall[1][jn])
                nc.scalar.dma_start(kt[:, 6:8, :], ktall[2][jn])
                if jn == 2:
                    # n=0 spmm chunks: queued behind kt_0..2 so the K.T tiles
                    # win the queue race; they land during phase 3.
                    for jq in range(4):
                        c = s8k.tile(
                            [128, 8, 512], BF16, tag="s8", name=f"x2c_0_{jq}"
                        )
                        eng = nc.scalar if jq % 2 == 0 else nc.gpsimd
                        eng.dma_start(c[:], x2n[0, :, jq * 8 : (jq + 1) * 8, :])
                        ch_first.append(c)
                for jl in range(4):
                    jj = jn * 4 + jl
                    lmc = lmcs[jl]
                    ps = accp.tile([128, 512], F32, tag="acc", name=f"ps3_{jj}")
                    for d in range(DC):
                        nc.tensor.matmul(
                            ps[:],
                            kt[:, d, jl * 128 : (jl + 1) * 128],
                            qt[:, d, :],
                            start=(d == 0),
                            stop=(d == DC - 1),
                        )
                    nc.vector.tensor_add(ps[:], ps[:], lmc[:, 0, :])
                    nc.scalar.activation(et[:, jj, :], ps[:], Exp)
                    pend.append(jj)
                    flush_esum(2)
            flush_esum(0)

            # ---- a_i = (interactions/topk + 1e-8) / (sum_e + 1e-8), column-major ----
            esr = persist.tile([1, 512], F32, tag="esr")
            nc.scalar.copy(esr[:], esum[:])
            ecol = persist.tile([128, IC, 1], F32, tag="ecol")
            for i in range(IC):
                pt = accp.tile([128, 1], F32, tag="acc", name=f"tr_{i}")
                nc.tensor.transpose(
                    pt[:], esr[:, i * 128 : (i + 1) * 128], ones_f[0:1, 0:1]
                )
                nc.scalar.copy(ecol[:, i, :], pt[:])
            rec = persist.tile([128, IC, 1], F32, tag="rec")
            nc.vector.tensor_scalar_add(ecol[:], ecol[:], 1e-8)
            nc.vector.reciprocal(rec[:], ecol[:])
            asb = persist.tile([128, IC, 1], F32, tag="asb")
            nc.vector.tensor_mul(asb[:], ia[:], rec[:])

            # ---- phase 4: spmm, one 32-matmul chain per output tile ----
            for n in range(TN):
                if n == 0:
                    ch = ch_first
                else:
                    ch = []
                    for jq in range(4):
                        c = s8k.tile([128, 8, 512], BF16, tag="s8", name=f"x2c_{n}_{jq}")
                        eng = nc.scalar if jq % 2 == 0 else nc.gpsimd
                        eng.dma_start(c[:], x2n[n, :, jq * 8 : (jq + 1) * 8, :])
                        ch.append(c)
                for i in range(IC):
                    ps = accp.tile([128, 512], F32, tag="acc", name=f"ps4_{n}_{i}")
                    for j in range(JC):
                        nc.tensor.matmul(
                            ps[:],
                            et[:, j, i * 128 : (i + 1) * 128],
                            ch[j // 8][:, j % 8, :],
                            start=(j == 0),
                            stop=(j == JC - 1),
                        )
                    o = outp.tile([128, 512], F32, tag="o")
                    if i % 2 == 0:
                        nc.vector.tensor_scalar_mul(o[:], ps[:], asb[:, i, :])
                    else:
                        nc.scalar.mul(o[:], ps[:], asb[:, i, :])
                    nc.sync.dma_start(
                        y[i * 128 : (i + 1) * 128, n * 512 : (n + 1) * 512], o[:]
                    )

    nc.compile()
    return nc


def _pmajor(a, p, inner):
    """[R, C] with R = nblk*p -> [p, nblk, C] partition-major, where each
    partition's inner block is contiguous."""
    R, C = a.shape
    nblk = R // p
    return np.ascontiguousarray(a.reshape(nblk, p, C).transpose(1, 0, 2))


def _in_maps(x1, x2, label_map, Wq, bq, Wk, DIMP, S, denom):
    ITEM = x1.shape[1]
    N2 = x2.shape[0]
    DIM = Wq.shape[0]
    DC = DIMP // 128
    TC = ITEM // 128
    JN = N2 // 512
    TN = ITEM // 512
    JC = N2 // 128
    IC = S // 128

    wqp = np.zeros((DIMP, ITEM), NPBF16)
    wqp[:DIM] = Wq.astype(NPBF16)
    wkp = np.zeros((DIMP, ITEM), NPBF16)
    wkp[:DIM] = Wk.astype(NPBF16)
    bqp = np.zeros((DIMP,), np.float32)
    bqp[:DIM] = bq / denom
    bq2 = np.ascontiguousarray(bqp.reshape(DC, 128).T)

    x1b = x1.astype(NPBF16)
    x2b = x2.astype(NPBF16)
    wqT = np.ascontiguousarray(wqp.T)  # [ITEM, DIMP]
    x2T = np.ascontiguousarray(x2b.T)  # [ITEM, N2]

    # wqt[d] = WqT[:, d-chunk] as [128, TC, 128] partition-major
    wqt = np.stack(
        [_pmajor(wqT[:, d * 128 : (d + 1) * 128], 128, None) for d in range(DC)]
    )
    wkT = np.ascontiguousarray(wkp.T)
    wktb = np.stack(
        [_pmajor(wkT[:, d * 128 : (d + 1) * 128], 128, None) for d in range(DC)]
    )
    # x2t[jn] = x2T[:, jn-chunk] as [128, TC, 512]
    x2tb = np.stack(
        [_pmajor(x2T[:, j * 512 : (j + 1) * 512], 128, None) for j in range(JN)]
    )
    # x2n[n] = x2[:, n-chunk] as [128, JC, 512]
    x2nb = np.stack(
        [_pmajor(x2b[:, n * 512 : (n + 1) * 512], 128, None) for n in range(TN)]
    )
    maps = []
    for c in range(NCORES):
        sl = slice(c * S, (c + 1) * S)
        shard = label_map[sl]
        # normal orientation (0/1) for interaction row sums
        lmb = np.stack(
            [
                _pmajor(shard.astype(NPBF16)[:, j * 512 : (j + 1) * 512], 128, None)
                for j in range(JN)
            ]
        )
        # transposed additive mask: 0 where label=1, -30 where label=0
        mt = ((shard.T.astype(np.float32) - 1.0) * 30.0).astype(NPBF16)  # [N2, S]
        lmtb = _pmajor(mt, 128, None)  # [128, JC, S]
        maps.append(
            {
                "x1t": _pmajor(np.ascontiguousarray(x1b[sl].T), 128, None),
                "wqt": wqt,
                "wkt": wktb,
                "x2m": x2tb[c],
                "x2n": x2nb,
                "lmt": lmtb,
                "lmn": lmb,
                "bq2": bq2,
            }
        )
    return maps


def _run(x1, x2, label_map, Wq, bq, Wk, bk, topk, trace=False):
    x1 = np.asarray(x1, np.float32)
    x2 = np.asarray(x2, np.float32)
    label_map = np.asarray(label_map, np.float32)
    Wq = np.asarray(Wq, np.float32)
    bq = np.asarray(bq, np.float32)
    Wk = np.asarray(Wk, np.float32)
    N1, ITEM = x1.shape
    N2 = x2.shape[0]
    DIM = Wq.shape[0]
    S = N1 // NCORES
    DIMP = ((DIM + 127) // 128) * 128
    denom = math.sqrt(ITEM)
    nc = _build(S, N2, ITEM, DIMP, denom, float(topk))
    maps = _in_maps(x1, x2, label_map, Wq, bq, Wk, DIMP, S, denom)
    res = run_bass_kernel_spmd(
        nc, maps, list(range(NCORES)), trace=trace, trace_cores=[0] if trace else None
    )
    out = np.concatenate([res.results[c]["y"] for c in range(NCORES)], axis=0)
    return out.astype(np.float32), res


def kernel(x1, x2, label_map, Wq, bq, Wk, bk, topk):
    out, _ = _run(x1, x2, label_map, Wq, bq, Wk, bk, topk)
    return out
